# revision 14
# baseline (speedup 1.0000x reference)
"""Trainium2 Bass kernel for MultiHeadSelfAttention with RoPE (bf16 path).

Problem: x[2, 2048, 1024] @ W_qkv[1024, 3072] -> rope(q,k) -> softmax(q k^T/8) v
         -> out @ W_out[1024, 1024].

Sharding (8 cores): batch (2-way) x head-group (4-way, 4 heads each).
Each core computes a partial output [2048, 1024] = attnout_heads @ W_out_rows;
host sums the 4 head-group partials per batch.

All matmuls run in bf16 (inputs pre-cast on host; FWL hides the weight loads),
accumulating in fp32 PSUM. Elementwise work is bf16 end-to-end so the DVE gets
its 2x packed mode. Measured-rel-err budget is 2e-2; bf16 lands ~6e-3.

On-core dataflow is fully "transposed" so the PE never needs a transpose:
  qT,kT[c, s] = sum_e W[e, c] * xT[e, s]   (lhsT = W slice, rhs = xT)
  rot = Mswap @ qT (PE), q' = qT*cos + rot*sin_signed (DVE)
  scores[sk, sq] per head via K=128 packing: [kT_A|kT_B] against zero-padded
  q ([q_A|0] / [0|q_B]); both heads' 512-col scores land in one [128, 1024]
  PSUM tile so a single ScalarE exp (scale=1/8 folded) serves the pair.
  attnT[sk, sq] -> oT[d, sq] += [v|1]^T attn (ones column gives the softmax
  denominator in row 64 for free); normalize via ones-outer-product broadcast
  + reciprocal + multiply; out_partial[s, e] = att_oT.T @ W_out_rows.
"""

import sys

if "/opt/trn_rl_repo" not in sys.path:
    sys.path.insert(0, "/opt/trn_rl_repo")

import numpy as np

B, S, E = 2, 2048, 1024
ATT = 1024
H = 16
D = 64
HG = 4            # head groups (cores per batch)
HPG = H // HG     # heads per core = 4
PAIRS = HPG // 2  # head pairs per core = 2
ROPE_THETA = 10000.0
N_CORES = 8

CH = 512              # sq chunk for the attention inner loop
N_CH = S // CH        # 4 chunks
N_SK = S // 128       # 16 sk tiles
EK = E // 128         # 8 contraction tiles over embedding dim
NSC = S // 512        # 4 s-chunks for xT staging

_BUILT = {}


def _build_program():
    import concourse.bacc as bacc
    import concourse.tile as tile
    import concourse.mybir as mybir

    f32 = mybir.dt.float32
    bf16 = mybir.dt.bfloat16
    AF = mybir.ActivationFunctionType

    nc = bacc.Bacc(
        "TRN2",
        target_bir_lowering=False,
        debug=False,
        enable_asserts=False,
        num_devices=N_CORES,
    )

    xT = nc.dram_tensor("xT", [E, S], bf16, kind="ExternalInput").ap()
    w_qk = nc.dram_tensor("w_qk", [E, 2 * HPG * D], bf16, kind="ExternalInput").ap()
    w_v = nc.dram_tensor("w_v", [E, HPG * D], bf16, kind="ExternalInput").ap()
    w_o = nc.dram_tensor("w_o", [HPG * D, E], bf16, kind="ExternalInput").ap()
    cos_t = nc.dram_tensor("cos_t", [128, S], bf16, kind="ExternalInput").ap()
    sin_t = nc.dram_tensor("sin_t", [128, S], bf16, kind="ExternalInput").ap()
    mswap = nc.dram_tensor("mswap", [128, 128], bf16, kind="ExternalInput").ap()
    out = nc.dram_tensor("out", [S, E], f32, kind="ExternalOutput").ap()

    with tile.TileContext(nc) as tc:
        with (
            tc.tile_pool(name="const", bufs=1) as constp,
            tc.tile_pool(name="qkT", bufs=1) as qkTp,
            tc.tile_pool(name="vsb", bufs=1) as vp,
            tc.tile_pool(name="attnout", bufs=1) as aop,
            tc.tile_pool(name="wo", bufs=1) as wop,
        ):
            msw_sb = constp.tile([128, 128], bf16, tag="msw")
            # ones row at partition 64 so its base matches the denominator
            # rhs operand oX[64:65] of the broadcast matmuls
            onesrow = constp.tile([65, 64], bf16, tag="onesrow")
            nc.gpsimd.memset(onesrow[64:65, :], 1.0)
            # ACT warmup: get the exp table-set load off the critical path
            warm = constp.tile([65, 16], bf16, tag="warm")
            nc.scalar.activation(warm[64:65, :], onesrow[64:65, 0:16], AF.Exp, scale=0.125)

            # k' per pair: [128, S] (rows 0:64 head A dims, 64:128 head B).
            # q' per pair split into two zero-padded [128, S] tensors so the
            # scores matmuls contract over the full K=128 (2-head packing):
            # qzlo = [q'_A | 0], qzhi = [0 | q'_B].
            qzlo = [qkTp.tile([128, S], bf16, tag=f"qzlo{g}", name=f"qzlo{g}") for g in range(PAIRS)]
            qzhi = [qkTp.tile([128, S], bf16, tag=f"qzhi{g}", name=f"qzhi{g}") for g in range(PAIRS)]
            kT = [qkTp.tile([128, S], bf16, tag=f"kT{g}", name=f"kT{g}") for g in range(PAIRS)]
            for g in range(PAIRS):
                nc.gpsimd.memset(qzlo[g][64:128, :], 0.0)
                nc.gpsimd.memset(qzhi[g][0:64, :], 0.0)
            # v natural + aug ones column, 4 heads: head h occupies cols
            # [65h, 65h+64) = v, col 65h+64 = ones (softmax-denominator row)
            v_c = vp.tile([128, N_SK, 4 * 65], bf16, tag="vc", name="vc")
            for h in range(4):
                nc.gpsimd.memset(v_c[:, :, 65 * h + 64], 1.0)
            # normalized attention output per pair [128 (pair dims), S]
            att_o = [aop.tile([128, S], bf16, tag=f"ao{g}", name=f"ao{g}") for g in range(PAIRS)]
            # W_out rows per pair
            wo_sb = [wop.tile([128, E], bf16, tag=f"wo{g}", name=f"wo{g}") for g in range(PAIRS)]

            with (
                tc.tile_pool(name="xt", bufs=NSC) as xtp,
                tc.tile_pool(name="wqk", bufs=1) as wqkp,
                tc.tile_pool(name="wv", bufs=1) as wvp,
                tc.tile_pool(name="ropes", bufs=3) as ropep,
                tc.tile_pool(name="trig", bufs=1) as trigp,
                tc.tile_pool(name="projps", bufs=3, space="PSUM") as pjp,
                tc.tile_pool(name="rotps", bufs=2, space="PSUM") as rtp,
                tc.tile_pool(name="vps", bufs=2, space="PSUM") as vpp,
            ):
                cos_sb = trigp.tile([128, S], bf16, tag="cos")
                sin_sb = trigp.tile([128, S], bf16, tag="sin")
                # Packed DMAs: fold the E=8x128 contraction tiles into the
                # free dim so the whole operand moves in ONE descriptor
                # (DMA issues serialize at ~650ns each on an HWDGE ring).
                # Issue on both rings: x chunks on sync, weights/trig on
                # scalar (idle until the first projection PSUM lands).
                # wqk_c[p, e, c'] = w_qk[128e + p, c']
                wqk_c = wqkp.tile([128, EK, 2 * HPG * D], bf16, tag="wqk")
                nc.scalar.dma_start(
                    wqk_c[:], w_qk.rearrange("(e p) c -> p e c", p=128)
                )
                # xt_c[c][p, e, s'] = xT[128e + p, 512c + s']
                xt_c = []
                for c in range(NSC):
                    t = xtp.tile([128, EK, 512], bf16, tag="xt")
                    nc.sync.dma_start(
                        t[:],
                        xT[:, 512 * c : 512 * (c + 1)].rearrange(
                            "(e p) s -> p e s", p=128
                        ),
                    )
                    xt_c.append(t)
                nc.scalar.dma_start(msw_sb[:], mswap[:])
                nc.scalar.dma_start(cos_sb[:], cos_t[:])
                nc.scalar.dma_start(sin_sb[:], sin_t[:])
                wv_c = wvp.tile([128, EK, HPG * D], bf16, tag="wv")
                nc.scalar.dma_start(
                    wv_c[:], w_v.rearrange("(e p) c -> p e c", p=128)
                )
                for g in range(PAIRS):
                    nc.scalar.dma_start(wo_sb[g][:], w_o[128 * g : 128 * (g + 1), :])

                rope_pend = []

                def rope_tail():
                    (g_, dest, sl, raw) = rope_pend.pop(0)
                    rp = rtp.tile([128, 512], f32, tag="rot")
                    nc.tensor.matmul(rp[:], msw_sb[:], raw[:], start=True, stop=True)
                    # keep DVE ops same-dtype bf16 so 2x packed mode engages
                    rps = ropep.tile([128, 512], bf16, tag="rps")
                    nc.scalar.copy(rps[:], rp[:])
                    t2 = ropep.tile([128, 512], bf16, tag="t2")
                    nc.vector.tensor_mul(t2[:], raw[:], cos_sb[:, sl])
                    t1 = ropep.tile([128, 512], bf16, tag="t1")
                    nc.vector.tensor_mul(t1[:], rps[:], sin_sb[:, sl])
                    if dest is None:
                        nc.vector.tensor_add(qzlo[g_][0:64, sl], t1[0:64, :], t2[0:64, :])
                        nc.vector.tensor_add(qzhi[g_][64:128, sl], t1[64:128, :], t2[64:128, :])
                    else:
                        nc.vector.tensor_add(dest[:, sl], t1[:], t2[:])

                def proj_qk(g):
                    # qT / kT projection + rope, chunked over s
                    for ti, dest in ((0, None), (1, kT[g])):
                        coff = ti * HPG * D + 128 * g
                        for c in range(NSC):
                            sl = slice(512 * c, 512 * (c + 1))
                            pp = pjp.tile([128, 512], f32, tag="pj")
                            for e in range(EK):
                                nc.tensor.matmul(
                                    pp[:],
                                    wqk_c[:, e, coff : coff + 128],
                                    xt_c[c][:, e, :],
                                    start=(e == 0),
                                    stop=(e == EK - 1),
                                )
                            raw = ropep.tile([128, 512], bf16, tag="raw")
                            nc.scalar.copy(raw[:], pp[:])
                            rope_pend.append((g, dest, sl, raw))
                            if len(rope_pend) > 1:
                                rope_tail()

                def proj_v(st):
                    vp_ps = vpp.tile([128, 2 * 128], f32, tag="vps")
                    for e in range(EK):
                        nc.tensor.matmul(
                            vp_ps[:],
                            xt_c[st // 4][:, e, 128 * (st % 4) : 128 * (st % 4 + 1)],
                            wv_c[:, e, :],
                            start=(e == 0),
                            stop=(e == EK - 1),
                        )
                    # single strided cast into the 4 head slots (skips ones col)
                    nc.vector.tensor_copy(
                        v_c[:, st, 0 : 4 * 65].rearrange("p (h d) -> p h d", h=4)[:, :, 0:64],
                        vp_ps[:].rearrange("p (h d) -> p h d", h=4),
                    )

                proj_qk(0)
                proj_qk(1)
                while rope_pend:
                    rope_tail()
                for st in range(N_SK):
                    proj_v(st)

            # ---------------- attention + output projection ----------------
            with (
                tc.tile_pool(name="attps", bufs=2, space="PSUM") as attps,
                tc.tile_pool(name="oTps", bufs=2, space="PSUM") as oTps,
                tc.tile_pool(name="expp", bufs=4) as expp,
                tc.tile_pool(name="recipp", bufs=2) as rcp,
                tc.tile_pool(name="osb", bufs=3) as osbp,
            ):
                def outproj(ch):
                    # output projection for a finished sq chunk
                    for st in range(CH * ch // 128, CH * (ch + 1) // 128):
                        ssl = slice(128 * st, 128 * (st + 1))
                        op = attps.tile([128, 1024], f32, tag="sAB", name=f"op{st}")
                        for g in range(PAIRS):
                            for n in range(E // 512):
                                nsl = slice(512 * n, 512 * (n + 1))
                                nc.tensor.matmul(
                                    op[:, nsl],
                                    att_o[g][:, ssl],
                                    wo_sb[g][:, nsl],
                                    start=(g == 0),
                                    stop=(g == PAIRS - 1),
                                )
                        ot = osbp.tile([128, E], f32, tag="ot")
                        nc.vector.tensor_copy(ot[:], op[:])
                        nc.sync.dma_start(out[ssl, :], ot[:])

                for ch in range(N_CH):
                    cslice = slice(CH * ch, CH * (ch + 1))
                    for g in range(PAIRS):
                        if g == 1 and ch > 0:
                            # previous chunk's output projection: emitted one
                            # unit late so its matmuls never wait on the
                            # normalize chain of the chunk they read
                            outproj(ch - 1)
                        hA, hB = 2 * g, 2 * g + 1
                        oTA = oTps.tile([65, CH], f32, tag="oTA")
                        oTB = oTps.tile([65, CH], f32, tag="oTB")
                        exps = []

                        def attnv(sk):
                            eAB = exps[sk]
                            first = sk == 0
                            last = sk == N_SK - 1
                            nc.tensor.matmul(
                                oTA[:],
                                v_c[:, sk, 65 * hA : 65 * hA + 65],
                                eAB[:, 0:512],
                                start=first,
                                stop=last,
                            )
                            nc.tensor.matmul(
                                oTB[:],
                                v_c[:, sk, 65 * hB : 65 * hB + 65],
                                eAB[:, 512:1024],
                                start=first,
                                stop=last,
                            )

                        for sk in range(N_SK):
                            sksl = slice(128 * sk, 128 * (sk + 1))
                            sAB = attps.tile([128, 1024], f32, tag="sAB")
                            nc.tensor.matmul(
                                sAB[:, 0:512], kT[g][:, sksl], qzlo[g][:, cslice],
                                start=True, stop=True,
                            )
                            nc.tensor.matmul(
                                sAB[:, 512:1024], kT[g][:, sksl], qzhi[g][:, cslice],
                                start=True, stop=True,
                            )
                            eAB = expp.tile([128, 1024], bf16, tag="eAB")
                            nc.scalar.activation(eAB[:], sAB[:], AF.Exp, scale=0.125)
                            exps.append(eAB)
                            if sk > 0:
                                attnv(sk - 1)
                        attnv(N_SK - 1)

                        # normalize: denominators live in row 64 of oTA/oTB.
                        # Broadcast across 64 partitions with a K=1 ones
                        # outer-product, recip, one multiply per head.
                        for head, oT in ((0, oTA), (1, oTB)):
                            # stage the denominator row to SBUF at partition 64
                            # (matches onesrow's base for the broadcast matmul)
                            oX = rcp.tile([65, CH], bf16, tag=f"oX{head}")
                            nc.vector.tensor_copy(oX[64:65, :], oT[64:65, :])
                            # db borrows a scores slot so it never blocks the
                            # next unit's oT allocation (attnv start)
                            db = attps.tile([64, CH], f32, tag="sAB")
                            nc.tensor.matmul(
                                db[:], onesrow[64:65, :], oX[64:65, :],
                                start=True, stop=True,
                            )
                            rb = rcp.tile([64, CH], f32, tag=f"rb{head}")
                            nc.vector.reciprocal_approx_fast(rb[:], db[:])
                            if head == 0:
                                nc.vector.tensor_mul(
                                    att_o[g][0:64, cslice], oT[0:64, :], rb[:]
                                )
                            else:
                                aoB = rcp.tile([64, CH], bf16, tag="aoB")
                                nc.vector.tensor_mul(aoB[:], oT[0:64, :], rb[:])
                                nc.sync.dma_start(att_o[g][64:128, cslice], aoB[:])

                outproj(N_CH - 2)
                outproj(N_CH - 1)

    nc.compile()
    return nc


def _get_program():
    if "nc" not in _BUILT:
        _BUILT["nc"] = _build_program()
    return _BUILT["nc"]


def _host_inputs(x, W_qkv, W_out):
    """Build the 8 per-core input maps (bf16)."""
    import ml_dtypes

    bf = ml_dtypes.bfloat16
    f = np.float32
    x = np.asarray(x, dtype=f)
    W_qkv = np.asarray(W_qkv, dtype=f)
    W_out = np.asarray(W_out, dtype=f)

    inv_freq = 1.0 / (ROPE_THETA ** (np.arange(0, D, 2, dtype=np.float64) / D))
    p = np.arange(128)
    freq_row = inv_freq[(p % D) // 2]  # [128]
    ang = freq_row[:, None] * np.arange(S, dtype=np.float64)[None, :]  # [128, S]
    cos_t = np.cos(ang).astype(bf)
    sign = np.where(p % 2 == 0, -1.0, 1.0)[:, None]
    sin_t = (np.sin(ang) * sign).astype(bf)

    msw = np.zeros((128, 128), dtype=bf)
    msw[p, p ^ 1] = 1.0

    maps = []
    for core in range(N_CORES):
        b, hg = divmod(core, HG)
        hs = [HPG * hg + i for i in range(HPG)]
        w_qk = np.concatenate(
            [W_qkv[:, h * D : (h + 1) * D] for h in hs]
            + [W_qkv[:, ATT + h * D : ATT + (h + 1) * D] for h in hs],
            axis=1,
        )
        w_v = np.concatenate(
            [W_qkv[:, 2 * ATT + h * D : 2 * ATT + (h + 1) * D] for h in hs], axis=1
        )
        w_o = np.concatenate([W_out[h * D : (h + 1) * D, :] for h in hs], axis=0)
        maps.append(
            {
                "xT": np.ascontiguousarray(x[b].T).astype(bf),
                "w_qk": np.ascontiguousarray(w_qk).astype(bf),
                "w_v": np.ascontiguousarray(w_v).astype(bf),
                "w_o": np.ascontiguousarray(w_o).astype(bf),
                "cos_t": cos_t,
                "sin_t": sin_t,
                "mswap": msw,
            }
        )
    return maps


def _gather(res, inputs=None):
    out = np.zeros((B, S, E), dtype=np.float32)
    for core in range(N_CORES):
        b = core // HG
        out[b] += res.results[core]["out"]
    return out


def kernel(x, W_qkv, W_out):
    from concourse.bass_utils import run_bass_kernel_spmd

    nc = _get_program()
    maps = _host_inputs(x, W_qkv, W_out)
    res = run_bass_kernel_spmd(nc, maps, core_ids=list(range(N_CORES)))
    return _gather(res)


# revision 15
# speedup vs baseline: 1.0613x; 1.0613x over previous
"""Trainium2 Bass kernel for MultiHeadSelfAttention with RoPE (bf16 path).

Problem: x[2, 2048, 1024] @ W_qkv[1024, 3072] -> rope(q,k) -> softmax(q k^T/8) v
         -> out @ W_out[1024, 1024].

Sharding (8 cores): batch (2-way) x head-group (4-way, 4 heads each).
Each core computes a partial output [2048, 1024] = attnout_heads @ W_out_rows;
host sums the 4 head-group partials per batch.

All matmuls run in bf16 (inputs pre-cast + DMA-layout-packed on host),
accumulating in fp32 PSUM; elementwise work is bf16 (DVE 2x packed mode).

Schedule: the attention inner loop is paced by ScalarE's exp (1114ns per
[128,1024] tile vs 860ns of PE work per sk tile), so all remaining PE work
is drained INTO those gaps as background:
  qk-proj(pair0) ; v-proj  ->  attention units (ch,pair0) x4 with
  qk-proj(pair1) interleaved as background  ->  units (ch,pair1) x4 with
  the output projection of finished chunks interleaved  ->  tail outproj.

On-core dataflow is fully "transposed" so the PE never needs a transpose:
  qT,kT[c, s] = sum_e W[e, c] * xT[e, s]   (lhsT = W slice, rhs = xT)
  rot = Mswap @ qT (PE), q' = qT*cos + rot*sin_signed (DVE)
  scores[sk, sq] per head via K=128 packing: [kT_A|kT_B] against zero-padded
  q ([q_A|0] / [0|q_B]); both heads' 512-col scores land in one [128, 1024]
  PSUM tile so a single ScalarE exp (scale=1/8 folded) serves the pair.
  attnT[sk, sq] -> oT[d, sq] += [v|1]^T attn (ones column gives the softmax
  denominator in row 64 for free); normalize via ones-outer-product broadcast
  + reciprocal + multiply; out_partial[s, e] = att_oT.T @ W_out_rows.
"""

import sys

if "/opt/trn_rl_repo" not in sys.path:
    sys.path.insert(0, "/opt/trn_rl_repo")

import numpy as np

B, S, E = 2, 2048, 1024
ATT = 1024
H = 16
D = 64
HG = 4            # head groups (cores per batch)
HPG = H // HG     # heads per core = 4
PAIRS = HPG // 2  # head pairs per core = 2
ROPE_THETA = 10000.0
N_CORES = 8

CH = 512              # sq chunk for the attention inner loop
N_CH = S // CH        # 4 chunks
N_SK = S // 128       # 16 sk tiles
EK = E // 128         # 8 contraction tiles over embedding dim
NSC = S // 512        # 4 s-chunks for xT staging

_BUILT = {}


def _build_program():
    import concourse.bacc as bacc
    import concourse.tile as tile
    import concourse.mybir as mybir

    f32 = mybir.dt.float32
    bf16 = mybir.dt.bfloat16
    AF = mybir.ActivationFunctionType

    nc = bacc.Bacc(
        "TRN2",
        target_bir_lowering=False,
        debug=False,
        enable_asserts=False,
        num_devices=N_CORES,
    )

    # All wide operands host-packed to [128, e, cols] so each is ONE
    # contiguous full-bandwidth DMA (row p = concat over e of tile rows).
    xTs = [
        nc.dram_tensor(f"xT{c}", [128, EK * 512], bf16, kind="ExternalInput").ap()
        for c in range(NSC)
    ]
    w_qk = nc.dram_tensor("w_qk", [128, EK * 2 * HPG * D], bf16, kind="ExternalInput").ap()
    w_v = nc.dram_tensor("w_v", [128, EK * HPG * D], bf16, kind="ExternalInput").ap()
    w_o = nc.dram_tensor("w_o", [HPG * D, E], bf16, kind="ExternalInput").ap()
    cos_t = nc.dram_tensor("cos_t", [128, S], bf16, kind="ExternalInput").ap()
    sin_t = nc.dram_tensor("sin_t", [128, S], bf16, kind="ExternalInput").ap()
    mswap = nc.dram_tensor("mswap", [128, 128], bf16, kind="ExternalInput").ap()
    out = nc.dram_tensor("out", [S, E], f32, kind="ExternalOutput").ap()

    with tile.TileContext(nc) as tc:
        with (
            tc.tile_pool(name="const", bufs=1) as constp,
            tc.tile_pool(name="qkT", bufs=1) as qkTp,
            tc.tile_pool(name="vsb", bufs=1) as vp,
            tc.tile_pool(name="attnout", bufs=1) as aop,
            tc.tile_pool(name="wo", bufs=1) as wop,
            tc.tile_pool(name="xt", bufs=NSC) as xtp,
            tc.tile_pool(name="wqk", bufs=1) as wqkp,
            tc.tile_pool(name="wv", bufs=1) as wvp,
            tc.tile_pool(name="ropes", bufs=3) as ropep,
            tc.tile_pool(name="trig", bufs=1) as trigp,
            tc.tile_pool(name="bgps", bufs=2, space="PSUM") as bgps,
            tc.tile_pool(name="attps", bufs=2, space="PSUM") as attps,
            tc.tile_pool(name="oTps", bufs=1, space="PSUM") as oTps,
            tc.tile_pool(name="expp", bufs=4) as expp,
            tc.tile_pool(name="recipp", bufs=2) as rcp,
            tc.tile_pool(name="osb", bufs=3) as osbp,
        ):
            msw_sb = constp.tile([128, 128], bf16, tag="msw")
            # ones row at partition 64 so its base matches the denominator
            # rhs operand oX[64:65] of the broadcast matmuls
            onesrow = constp.tile([65, 64], bf16, tag="onesrow")
            nc.gpsimd.memset(onesrow[64:65, :], 1.0)
            # ACT warmup: get the exp table-set load off the critical path
            warm = constp.tile([65, 16], bf16, tag="warm")
            nc.scalar.activation(warm[64:65, :], onesrow[64:65, 0:16], AF.Exp, scale=0.125)

            qzlo = [qkTp.tile([128, S], bf16, tag=f"qzlo{g}", name=f"qzlo{g}") for g in range(PAIRS)]
            qzhi = [qkTp.tile([128, S], bf16, tag=f"qzhi{g}", name=f"qzhi{g}") for g in range(PAIRS)]
            kT = [qkTp.tile([128, S], bf16, tag=f"kT{g}", name=f"kT{g}") for g in range(PAIRS)]
            for g in range(PAIRS):
                nc.gpsimd.memset(qzlo[g][64:128, :], 0.0)
                nc.gpsimd.memset(qzhi[g][0:64, :], 0.0)
            # v natural + aug ones column, 4 heads: head h occupies cols
            # [65h, 65h+64) = v, col 65h+64 = ones (softmax-denominator row)
            v_c = vp.tile([128, N_SK, 4 * 65], bf16, tag="vc", name="vc")
            for h in range(4):
                nc.gpsimd.memset(v_c[:, :, 65 * h + 64], 1.0)
            att_o = [aop.tile([128, S], bf16, tag=f"ao{g}", name=f"ao{g}") for g in range(PAIRS)]
            wo_sb = [wop.tile([128, E], bf16, tag=f"wo{g}", name=f"wo{g}") for g in range(PAIRS)]

            cos_sb = trigp.tile([128, S], bf16, tag="cos")
            sin_sb = trigp.tile([128, S], bf16, tag="sin")
            # x chunks on the sync HWDGE ring, everything else on scalar's
            wqk_c = wqkp.tile([128, EK, 2 * HPG * D], bf16, tag="wqk")
            nc.scalar.dma_start(wqk_c[:], w_qk.rearrange("p (e c) -> p e c", e=EK))
            xt_c = []
            for c in range(NSC):
                t = xtp.tile([128, EK, 512], bf16, tag="xt")
                nc.sync.dma_start(t[:], xTs[c].rearrange("p (e s) -> p e s", e=EK))
                xt_c.append(t)
            nc.scalar.dma_start(msw_sb[:], mswap[:])
            nc.scalar.dma_start(cos_sb[:], cos_t[:])
            nc.scalar.dma_start(sin_sb[:], sin_t[:])
            wv_c = wvp.tile([128, EK, HPG * D], bf16, tag="wv")
            nc.scalar.dma_start(wv_c[:], w_v.rearrange("p (e c) -> p e c", e=EK))
            for g in range(PAIRS):
                nc.scalar.dma_start(wo_sb[g][:], w_o[128 * g : 128 * (g + 1), :])

            # ---------------- emission helpers ----------------
            rope_pend = []

            def rope_tail():
                (g_, dest, sl, raw) = rope_pend.pop(0)
                rp = bgps.tile([128, 512], f32, tag="bg")
                nc.tensor.matmul(rp[:], msw_sb[:], raw[:], start=True, stop=True)
                rps = ropep.tile([128, 512], bf16, tag="rps")
                nc.scalar.copy(rps[:], rp[:])
                t2 = ropep.tile([128, 512], bf16, tag="t2")
                nc.vector.tensor_mul(t2[:], raw[:], cos_sb[:, sl])
                t1 = ropep.tile([128, 512], bf16, tag="t1")
                nc.vector.tensor_mul(t1[:], rps[:], sin_sb[:, sl])
                if dest is None:
                    nc.vector.tensor_add(qzlo[g_][0:64, sl], t1[0:64, :], t2[0:64, :])
                    nc.vector.tensor_add(qzhi[g_][64:128, sl], t1[64:128, :], t2[64:128, :])
                else:
                    nc.vector.tensor_add(dest[:, sl], t1[:], t2[:])

            def proj_qk_chain(g, ti, c):
                # one 8-matmul projection chain + queued rope tail
                dest = None if ti == 0 else kT[g]
                coff = ti * HPG * D + 128 * g
                sl = slice(512 * c, 512 * (c + 1))
                pp = bgps.tile([128, 512], f32, tag="bg")
                for e in range(EK):
                    nc.tensor.matmul(
                        pp[:],
                        wqk_c[:, e, coff : coff + 128],
                        xt_c[c][:, e, :],
                        start=(e == 0),
                        stop=(e == EK - 1),
                    )
                raw = ropep.tile([128, 512], bf16, tag="raw")
                nc.scalar.copy(raw[:], pp[:])
                rope_pend.append((g, dest, sl, raw))
                if len(rope_pend) > 1:
                    rope_tail()

            def proj_v(st):
                vp_ps = bgps.tile([128, 2 * 128], f32, tag="bg")
                for e in range(EK):
                    nc.tensor.matmul(
                        vp_ps[:],
                        xt_c[st // 4][:, e, 128 * (st % 4) : 128 * (st % 4 + 1)],
                        wv_c[:, e, :],
                        start=(e == 0),
                        stop=(e == EK - 1),
                    )
                nc.vector.tensor_copy(
                    v_c[:, st, 0 : 4 * 65].rearrange("p (h d) -> p h d", h=4)[:, :, 0:64],
                    vp_ps[:].rearrange("p (h d) -> p h d", h=4),
                )

            def outproj_st(st):
                ssl = slice(128 * st, 128 * (st + 1))
                op = attps.tile([128, 1024], f32, tag="sAB", name=f"op{st}")
                for g in range(PAIRS):
                    for n in range(E // 512):
                        nsl = slice(512 * n, 512 * (n + 1))
                        nc.tensor.matmul(
                            op[:, nsl],
                            att_o[g][:, ssl],
                            wo_sb[g][:, nsl],
                            start=(g == 0),
                            stop=(g == PAIRS - 1),
                        )
                ot = osbp.tile([128, E], f32, tag="ot")
                nc.vector.tensor_copy(ot[:], op[:])
                nc.sync.dma_start(out[ssl, :], ot[:])

            def attention_unit(g, ch, bg):
                """One (pair, chunk) unit. bg: {sk: [closures]} of background
                PE work drained into the exp-paced gaps."""
                cslice = slice(CH * ch, CH * (ch + 1))
                hA, hB = 2 * g, 2 * g + 1
                oTA = oTps.tile([65, CH], f32, tag="oTA")
                oTB = oTps.tile([65, CH], f32, tag="oTB")
                exps = []

                def attnv(sk):
                    eAB = exps[sk]
                    nc.tensor.matmul(
                        oTA[:],
                        v_c[:, sk, 65 * hA : 65 * hA + 65],
                        eAB[:, 0:512],
                        start=(sk == 0),
                        stop=(sk == N_SK - 1),
                    )
                    nc.tensor.matmul(
                        oTB[:],
                        v_c[:, sk, 65 * hB : 65 * hB + 65],
                        eAB[:, 512:1024],
                        start=(sk == 0),
                        stop=(sk == N_SK - 1),
                    )

                for sk in range(N_SK):
                    sksl = slice(128 * sk, 128 * (sk + 1))
                    sAB = attps.tile([128, 1024], f32, tag="sAB")
                    nc.tensor.matmul(
                        sAB[:, 0:512], kT[g][:, sksl], qzlo[g][:, cslice],
                        start=True, stop=True,
                    )
                    nc.tensor.matmul(
                        sAB[:, 512:1024], kT[g][:, sksl], qzhi[g][:, cslice],
                        start=True, stop=True,
                    )
                    eAB = expp.tile([128, 1024], bf16, tag="eAB")
                    nc.scalar.activation(eAB[:], sAB[:], AF.Exp, scale=0.125)
                    exps.append(eAB)
                    if sk > 0:
                        attnv(sk - 1)
                    for work in bg.get(sk, ()):
                        work()
                attnv(N_SK - 1)

                # normalize: denominators live in row 64 of oTA/oTB
                for head, oT in ((0, oTA), (1, oTB)):
                    oX = rcp.tile([65, CH], bf16, tag=f"oX{head}")
                    nc.vector.tensor_copy(oX[64:65, :], oT[64:65, :])
                    db = attps.tile([64, CH], f32, tag="sAB")
                    nc.tensor.matmul(
                        db[:], onesrow[64:65, :], oX[64:65, :],
                        start=True, stop=True,
                    )
                    rb = rcp.tile([64, CH], f32, tag=f"rb{head}")
                    nc.vector.reciprocal_approx_fast(rb[:], db[:])
                    if head == 0:
                        nc.vector.tensor_mul(
                            att_o[g][0:64, cslice], oT[0:64, :], rb[:]
                        )
                    else:
                        aoB = rcp.tile([64, CH], bf16, tag="aoB")
                        nc.vector.tensor_mul(aoB[:], oT[0:64, :], rb[:])
                        nc.sync.dma_start(att_o[g][64:128, cslice], aoB[:])

            # ---------------- emission ----------------
            # serial head: pair-0 projection + all of v
            for ti in range(2):
                for c in range(NSC):
                    proj_qk_chain(0, ti, c)
            while rope_pend:
                rope_tail()
            for st in range(N_SK):
                proj_v(st)

            # pair-0 attention, with pair-1 projection as background
            # (2 chains per unit at sk 5 and 11 -> all 8 chains + tails
            #  drain across the four pair-0 units)
            g1_chains = [(1, ti, c) for ti in range(2) for c in range(NSC)]
            for ch in range(N_CH):
                i = 2 * ch
                bg = {
                    5: [lambda a=g1_chains[i]: proj_qk_chain(*a)],
                    11: [lambda a=g1_chains[i + 1]: proj_qk_chain(*a)],
                }
                attention_unit(0, ch, bg)
            while rope_pend:
                rope_tail()

            # pair-1 attention, with finished chunks' output projection as
            # background (chunk ch-1 is complete once unit (1, ch-1) is done)
            for ch in range(N_CH):
                if ch > 0:
                    sts = range(CH * (ch - 1) // 128, CH * ch // 128)
                    bg = {3 + 4 * j: [lambda s=st: outproj_st(s)]
                          for j, st in enumerate(sts)}
                else:
                    bg = {}
                attention_unit(1, ch, bg)
            for st in range(CH * (N_CH - 1) // 128, S // 128):
                outproj_st(st)

    nc.compile()
    return nc


def _get_program():
    if "nc" not in _BUILT:
        _BUILT["nc"] = _build_program()
    return _BUILT["nc"]


def _pack_e(a):
    """[E, C] -> [128, EK*C] with row p = concat over e of a[128e+p, :]."""
    Edim, C = a.shape
    return np.ascontiguousarray(
        a.reshape(EK, 128, C).transpose(1, 0, 2).reshape(128, EK * C)
    )


def _host_inputs(x, W_qkv, W_out):
    """Build the 8 per-core input maps (bf16, DMA-packed)."""
    import ml_dtypes

    bf = ml_dtypes.bfloat16
    f = np.float32
    x = np.asarray(x, dtype=f)
    W_qkv = np.asarray(W_qkv, dtype=f)
    W_out = np.asarray(W_out, dtype=f)

    inv_freq = 1.0 / (ROPE_THETA ** (np.arange(0, D, 2, dtype=np.float64) / D))
    p = np.arange(128)
    freq_row = inv_freq[(p % D) // 2]  # [128]
    ang = freq_row[:, None] * np.arange(S, dtype=np.float64)[None, :]  # [128, S]
    cos_t = np.cos(ang).astype(bf)
    sign = np.where(p % 2 == 0, -1.0, 1.0)[:, None]
    sin_t = (np.sin(ang) * sign).astype(bf)

    msw = np.zeros((128, 128), dtype=bf)
    msw[p, p ^ 1] = 1.0

    maps = []
    for core in range(N_CORES):
        b, hg = divmod(core, HG)
        hs = [HPG * hg + i for i in range(HPG)]
        w_qk = np.concatenate(
            [W_qkv[:, h * D : (h + 1) * D] for h in hs]
            + [W_qkv[:, ATT + h * D : ATT + (h + 1) * D] for h in hs],
            axis=1,
        )
        w_v = np.concatenate(
            [W_qkv[:, 2 * ATT + h * D : 2 * ATT + (h + 1) * D] for h in hs], axis=1
        )
        w_o = np.concatenate([W_out[h * D : (h + 1) * D, :] for h in hs], axis=0)
        xT = np.ascontiguousarray(x[b].T)
        m = {
            "w_qk": _pack_e(w_qk).astype(bf),
            "w_v": _pack_e(w_v).astype(bf),
            "w_o": np.ascontiguousarray(w_o).astype(bf),
            "cos_t": cos_t,
            "sin_t": sin_t,
            "mswap": msw,
        }
        for c in range(NSC):
            m[f"xT{c}"] = _pack_e(xT[:, 512 * c : 512 * (c + 1)]).astype(bf)
        maps.append(m)
    return maps


def _gather(res, inputs=None):
    out = np.zeros((B, S, E), dtype=np.float32)
    for core in range(N_CORES):
        b = core // HG
        out[b] += res.results[core]["out"]
    return out


def kernel(x, W_qkv, W_out):
    from concourse.bass_utils import run_bass_kernel_spmd

    nc = _get_program()
    maps = _host_inputs(x, W_qkv, W_out)
    res = run_bass_kernel_spmd(nc, maps, core_ids=list(range(N_CORES)))
    return _gather(res)


# revision 21
# speedup vs baseline: 1.1675x; 1.1001x over previous
"""Trainium2 Bass kernel for MultiHeadSelfAttention with RoPE (bf16 path).

Problem: x[2, 2048, 1024] @ W_qkv[1024, 3072] -> rope(q,k) -> softmax(q k^T/8) v
         -> out @ W_out[1024, 1024].

Sharding (8 cores): batch (2-way) x head-group (4-way, 4 heads each).
Each core computes a partial output [2048, 1024] = attnout_heads @ W_out_rows;
host sums the 4 head-group partials per batch.

All matmuls run in bf16 (inputs pre-cast + DMA-layout-packed on host),
accumulating in fp32 PSUM; elementwise work is bf16 (DVE 2x packed mode).

Schedule: the attention inner loop is paced by ScalarE's exp (1114ns per
[128,1024] tile vs 860ns of PE work per sk tile), so all remaining PE work
is drained INTO those gaps as background:
  qk-proj(pair0) ; v-proj  ->  attention units (ch,pair0) x4 with
  qk-proj(pair1) interleaved as background  ->  units (ch,pair1) x4 with
  the output projection of finished chunks interleaved  ->  tail outproj.

On-core dataflow is fully "transposed" so the PE never needs a transpose:
  qT,kT[c, s] = sum_e W[e, c] * xT[e, s]   (lhsT = W slice, rhs = xT)
  rot = Mswap @ qT (PE), q' = qT*cos + rot*sin_signed (DVE)
  scores[sk, sq] per head via K=128 packing: [kT_A|kT_B] against zero-padded
  q ([q_A|0] / [0|q_B]); both heads' 512-col scores land in one [128, 1024]
  PSUM tile so a single ScalarE exp (scale=1/8 folded) serves the pair.
  attnT[sk, sq] -> oT[d, sq] += [v|1]^T attn (ones column gives the softmax
  denominator in row 64 for free); normalize via ones-outer-product broadcast
  + reciprocal + multiply; out_partial[s, e] = att_oT.T @ W_out_rows.
"""

import sys

if "/opt/trn_rl_repo" not in sys.path:
    sys.path.insert(0, "/opt/trn_rl_repo")

import numpy as np

B, S, E = 2, 2048, 1024
ATT = 1024
H = 16
D = 64
HG = 4            # head groups (cores per batch)
HPG = H // HG     # heads per core = 4
PAIRS = HPG // 2  # head pairs per core = 2
ROPE_THETA = 10000.0
N_CORES = 8

CH = 512              # sq chunk for the attention inner loop
N_CH = S // CH        # 4 chunks
N_SK = S // 128       # 16 sk tiles
EK = E // 128         # 8 contraction tiles over embedding dim
NSC = S // 512        # 4 s-chunks for xT staging

_BUILT = {}


def _build_program():
    import concourse.bacc as bacc
    import concourse.tile as tile
    import concourse.mybir as mybir

    f32 = mybir.dt.float32
    bf16 = mybir.dt.bfloat16
    AF = mybir.ActivationFunctionType

    nc = bacc.Bacc(
        "TRN2",
        target_bir_lowering=False,
        debug=False,
        enable_asserts=False,
        num_devices=N_CORES,
    )

    # All wide operands host-packed to [128, e, cols] so each is ONE
    # contiguous full-bandwidth DMA (row p = concat over e of tile rows).
    xTs = [
        nc.dram_tensor(f"xT{c}", [128, EK * 512], bf16, kind="ExternalInput").ap()
        for c in range(NSC)
    ]
    w_qk = nc.dram_tensor("w_qk", [128, EK * 2 * HPG * D], bf16, kind="ExternalInput").ap()
    w_v = nc.dram_tensor("w_v", [128, EK * HPG * D], bf16, kind="ExternalInput").ap()
    w_o = nc.dram_tensor("w_o", [HPG * D, E], bf16, kind="ExternalInput").ap()
    cos_t = nc.dram_tensor("cos_t", [128, S], bf16, kind="ExternalInput").ap()
    sin_t = nc.dram_tensor("sin_t", [128, S], bf16, kind="ExternalInput").ap()
    mswap = nc.dram_tensor("mswap", [128, 128], bf16, kind="ExternalInput").ap()
    out = nc.dram_tensor("out", [S, E], f32, kind="ExternalOutput").ap()

    with tile.TileContext(nc) as tc:
        with (
            tc.tile_pool(name="const", bufs=1) as constp,
            tc.tile_pool(name="qkT", bufs=1) as qkTp,
            tc.tile_pool(name="vsb", bufs=1) as vp,
            tc.tile_pool(name="attnout", bufs=1) as aop,
            tc.tile_pool(name="wo", bufs=1) as wop,
            tc.tile_pool(name="xt", bufs=NSC) as xtp,
            tc.tile_pool(name="wqk", bufs=1) as wqkp,
            tc.tile_pool(name="wv", bufs=1) as wvp,
            tc.tile_pool(name="ropes", bufs=3) as ropep,
            tc.tile_pool(name="trig", bufs=1) as trigp,
            tc.tile_pool(name="bgps", bufs=2, space="PSUM") as bgps,
            tc.tile_pool(name="attps", bufs=2, space="PSUM") as attps,
            tc.tile_pool(name="oTps", bufs=1, space="PSUM") as oTps,
            tc.tile_pool(name="expp", bufs=4) as expp,
            tc.tile_pool(name="recipp", bufs=2) as rcp,
            tc.tile_pool(name="osb", bufs=3) as osbp,
        ):
            msw_sb = constp.tile([128, 128], bf16, tag="msw")
            # ones row at partition 64 so its base matches the denominator
            # rhs operand oX[64:65] of the broadcast matmuls
            onesrow = constp.tile([65, 64], bf16, tag="onesrow")
            nc.gpsimd.memset(onesrow[64:65, :], 1.0)
            # ACT warmup: get the exp table-set load off the critical path
            warm = constp.tile([65, 16], bf16, tag="warm")
            nc.scalar.activation(warm[64:65, :], onesrow[64:65, 0:16], AF.Exp, scale=0.125)

            qzlo = [qkTp.tile([128, S], bf16, tag=f"qzlo{g}", name=f"qzlo{g}") for g in range(PAIRS)]
            qzhi = [qkTp.tile([128, S], bf16, tag=f"qzhi{g}", name=f"qzhi{g}") for g in range(PAIRS)]
            kT = [qkTp.tile([128, S], bf16, tag=f"kT{g}", name=f"kT{g}") for g in range(PAIRS)]
            for g in range(PAIRS):
                nc.gpsimd.memset(qzlo[g][64:128, :], 0.0)
                nc.gpsimd.memset(qzhi[g][0:64, :], 0.0)
            # v natural + aug ones column, 4 heads: head h occupies cols
            # [65h, 65h+64) = v, col 65h+64 = ones (softmax-denominator row)
            v_c = vp.tile([128, N_SK, 4 * 65], bf16, tag="vc", name="vc")
            for h in range(4):
                nc.gpsimd.memset(v_c[:, :, 65 * h + 64], 1.0)
            att_o = [aop.tile([128, S], bf16, tag=f"ao{g}", name=f"ao{g}") for g in range(PAIRS)]
            wo_sb = [wop.tile([128, E], bf16, tag=f"wo{g}", name=f"wo{g}") for g in range(PAIRS)]

            cos_sb = trigp.tile([128, S], bf16, tag="cos")
            sin_sb = trigp.tile([128, S], bf16, tag="sin")
            # x chunks on the sync HWDGE ring, everything else on scalar's.
            # wqk lands in two pieces so the first projection chain (cols
            # 0:128) starts as soon as ~256KB arrive.
            wqk_c = wqkp.tile([128, EK, 2 * HPG * D], bf16, tag="wqk")
            wqk_r = w_qk.rearrange("p (e c) -> p e c", e=EK)
            nc.scalar.dma_start(wqk_c[:, :, 0:128], wqk_r[:, :, 0:128])
            nc.scalar.dma_start(wqk_c[:, :, 128:], wqk_r[:, :, 128:])
            xt_c = []
            for c in range(NSC):
                t = xtp.tile([128, EK, 512], bf16, tag="xt")
                nc.sync.dma_start(t[:], xTs[c].rearrange("p (e s) -> p e s", e=EK))
                xt_c.append(t)
            nc.scalar.dma_start(msw_sb[:], mswap[:])
            nc.scalar.dma_start(cos_sb[:], cos_t[:])
            nc.scalar.dma_start(sin_sb[:], sin_t[:])
            wv_c = wvp.tile([128, EK, HPG * D], bf16, tag="wv")
            nc.scalar.dma_start(wv_c[:], w_v.rearrange("p (e c) -> p e c", e=EK))
            for g in range(PAIRS):
                nc.scalar.dma_start(wo_sb[g][:], w_o[128 * g : 128 * (g + 1), :])

            # ---------------- emission helpers ----------------
            rope_pend = []

            def rope_tail():
                (g_, dest, sl, raw, ptag) = rope_pend.pop(0)
                rp = (bgps if ptag == "bg" else attps).tile(
                    [128, 512], f32, tag=ptag
                )
                nc.tensor.matmul(rp[:], msw_sb[:], raw[:], start=True, stop=True)
                rps = ropep.tile([128, 512], bf16, tag="rps")
                nc.scalar.copy(rps[:], rp[:])
                t2 = ropep.tile([128, 512], bf16, tag="t2")
                nc.vector.tensor_mul(t2[:], raw[:], cos_sb[:, sl])
                t1 = ropep.tile([128, 512], bf16, tag="t1")
                nc.vector.tensor_mul(t1[:], rps[:], sin_sb[:, sl])
                if dest is None:
                    nc.vector.tensor_add(qzlo[g_][0:64, sl], t1[0:64, :], t2[0:64, :])
                    nc.vector.tensor_add(qzhi[g_][64:128, sl], t1[64:128, :], t2[64:128, :])
                else:
                    nc.vector.tensor_add(dest[:, sl], t1[:], t2[:])

            def proj_qk_chain(g, ti, c, ptag="bg"):
                # one 8-matmul projection chain + queued rope tail.
                # During the serial head, chains alternate between the bg
                # PSUM slot and the (still idle) scores slot so consecutive
                # chains double-buffer; background chains inside attention
                # always use the bg slot.
                dest = None if ti == 0 else kT[g]
                coff = ti * HPG * D + 128 * g
                sl = slice(512 * c, 512 * (c + 1))
                pp = (bgps if ptag == "bg" else attps).tile(
                    [128, 512], f32, tag=ptag
                )
                for e in range(EK):
                    nc.tensor.matmul(
                        pp[:],
                        wqk_c[:, e, coff : coff + 128],
                        xt_c[c][:, e, :],
                        start=(e == 0),
                        stop=(e == EK - 1),
                    )
                raw = ropep.tile([128, 512], bf16, tag="raw")
                nc.scalar.copy(raw[:], pp[:])
                rope_pend.append((g, dest, sl, raw, ptag))
                if len(rope_pend) > 1:
                    rope_tail()

            def proj_v(st):
                vp_ps = bgps.tile([128, 2 * 128], f32, tag="bg")
                for e in range(EK):
                    nc.tensor.matmul(
                        vp_ps[:],
                        xt_c[st // 4][:, e, 128 * (st % 4) : 128 * (st % 4 + 1)],
                        wv_c[:, e, :],
                        start=(e == 0),
                        stop=(e == EK - 1),
                    )
                nc.vector.tensor_copy(
                    v_c[:, st, 0 : 4 * 65].rearrange("p (h d) -> p h d", h=4)[:, :, 0:64],
                    vp_ps[:].rearrange("p (h d) -> p h d", h=4),
                )

            def outproj_st(st):
                # two 512-wide halves through the 1-bank bg slots so the
                # scores double-buffer (sAB) is never disturbed
                ssl = slice(128 * st, 128 * (st + 1))
                ot = osbp.tile([128, E], f32, tag="ot")
                for n in range(E // 512):
                    nsl = slice(512 * n, 512 * (n + 1))
                    op = bgps.tile([128, 512], f32, tag="bg")
                    for g in range(PAIRS):
                        nc.tensor.matmul(
                            op[:],
                            att_o[g][:, ssl],
                            wo_sb[g][:, nsl],
                            start=(g == 0),
                            stop=(g == PAIRS - 1),
                        )
                    nc.vector.tensor_copy(ot[:, nsl], op[:])
                nc.sync.dma_start(out[ssl, :], ot[:])

            def attention_unit(g, ch, bg):
                """One (pair, chunk) unit. bg: {sk: [closures]} of background
                PE work drained into the exp-paced gaps."""
                cslice = slice(CH * ch, CH * (ch + 1))
                hA, hB = 2 * g, 2 * g + 1
                oTA = oTps.tile([65, CH], f32, tag="oTA")
                oTB = oTps.tile([65, CH], f32, tag="oTB")
                exps = []

                def attnv(sk):
                    eAB = exps[sk]
                    nc.tensor.matmul(
                        oTA[:],
                        v_c[:, sk, 65 * hA : 65 * hA + 65],
                        eAB[:, 0:512],
                        start=(sk == 0),
                        stop=(sk == N_SK - 1),
                    )
                    nc.tensor.matmul(
                        oTB[:],
                        v_c[:, sk, 65 * hB : 65 * hB + 65],
                        eAB[:, 512:1024],
                        start=(sk == 0),
                        stop=(sk == N_SK - 1),
                    )

                for sk in range(N_SK):
                    sksl = slice(128 * sk, 128 * (sk + 1))
                    sAB = attps.tile([128, 1024], f32, tag="sAB")
                    nc.tensor.matmul(
                        sAB[:, 0:512], kT[g][:, sksl], qzlo[g][:, cslice],
                        start=True, stop=True,
                    )
                    nc.tensor.matmul(
                        sAB[:, 512:1024], kT[g][:, sksl], qzhi[g][:, cslice],
                        start=True, stop=True,
                    )
                    eAB = expp.tile([128, 1024], bf16, tag="eAB")
                    nc.scalar.activation(eAB[:], sAB[:], AF.Exp, scale=0.125)
                    exps.append(eAB)
                    if sk > 0:
                        attnv(sk - 1)
                    for work in bg.get(sk, ()):
                        work()
                attnv(N_SK - 1)

                # normalize: denominators live in row 64 of oTA/oTB.
                # Head-interleaved, denom-row staging on ScalarE (idle at
                # unit boundaries), db in the bg slot — so neither the next
                # unit's scores (sAB) nor its attnv (oT) wait on this chain
                # longer than necessary.
                oXA = rcp.tile([65, CH], bf16, tag="oX0")
                oXB = rcp.tile([65, CH], bf16, tag="oX1")
                nc.scalar.copy(oXA[64:65, :], oTA[64:65, :])
                nc.scalar.copy(oXB[64:65, :], oTB[64:65, :])
                dbA = bgps.tile([64, CH], f32, tag="bg")
                dbB = bgps.tile([64, CH], f32, tag="bg")
                nc.tensor.matmul(
                    dbA[:], onesrow[64:65, :], oXA[64:65, :], start=True, stop=True
                )
                nc.tensor.matmul(
                    dbB[:], onesrow[64:65, :], oXB[64:65, :], start=True, stop=True
                )
                rbA = rcp.tile([64, CH], f32, tag="rb0")
                rbB = rcp.tile([64, CH], f32, tag="rb1")
                nc.vector.reciprocal_approx_fast(rbA[:], dbA[:])
                nc.vector.reciprocal_approx_fast(rbB[:], dbB[:])
                nc.vector.tensor_mul(att_o[g][0:64, cslice], oTA[0:64, :], rbA[:])
                aoB = rcp.tile([64, CH], bf16, tag="aoB")
                nc.vector.tensor_mul(aoB[:], oTB[0:64, :], rbB[:])
                nc.sync.dma_start(att_o[g][64:128, cslice], aoB[:])

            # ---------------- emission ----------------
            # serial head: pair-0 projection + all of v (chains alternate
            # PSUM slots between the bg tag and the idle scores tag)
            for i, (ti, c) in enumerate([(t, c) for t in range(2) for c in range(NSC)]):
                proj_qk_chain(0, ti, c, ptag=("bg", "sAB")[i % 2])
            while rope_pend:
                rope_tail()
            for st in range(N_SK):
                proj_v(st)

            # pair-0 attention, with pair-1 projection as background
            # (2 chains per unit at sk 5 and 11 -> all 8 chains + tails
            #  drain across the four pair-0 units)
            g1_chains = [(1, ti, c) for ti in range(2) for c in range(NSC)]
            for ch in range(N_CH):
                i = 2 * ch
                bg = {
                    5: [lambda a=g1_chains[i]: proj_qk_chain(*a)],
                    11: [lambda a=g1_chains[i + 1]: proj_qk_chain(*a)],
                }
                attention_unit(0, ch, bg)
            while rope_pend:
                rope_tail()

            # pair-1 attention, with finished chunks' output projection as
            # background (chunk ch-1 is complete once unit (1, ch-1) is done)
            for ch in range(N_CH):
                if ch > 0:
                    sts = range(CH * (ch - 1) // 128, CH * ch // 128)
                    bg = {3 + 4 * j: [lambda s=st: outproj_st(s)]
                          for j, st in enumerate(sts)}
                else:
                    bg = {}
                attention_unit(1, ch, bg)
            for st in range(CH * (N_CH - 1) // 128, S // 128):
                outproj_st(st)

    nc.compile()
    return nc


def _get_program():
    if "nc" not in _BUILT:
        _BUILT["nc"] = _build_program()
    return _BUILT["nc"]


def _pack_e(a):
    """[E, C] -> [128, EK*C] with row p = concat over e of a[128e+p, :]."""
    Edim, C = a.shape
    return np.ascontiguousarray(
        a.reshape(EK, 128, C).transpose(1, 0, 2).reshape(128, EK * C)
    )


def _host_inputs(x, W_qkv, W_out):
    """Build the 8 per-core input maps (bf16, DMA-packed)."""
    import ml_dtypes

    bf = ml_dtypes.bfloat16
    f = np.float32
    x = np.asarray(x, dtype=f)
    W_qkv = np.asarray(W_qkv, dtype=f)
    W_out = np.asarray(W_out, dtype=f)

    inv_freq = 1.0 / (ROPE_THETA ** (np.arange(0, D, 2, dtype=np.float64) / D))
    p = np.arange(128)
    freq_row = inv_freq[(p % D) // 2]  # [128]
    ang = freq_row[:, None] * np.arange(S, dtype=np.float64)[None, :]  # [128, S]
    cos_t = np.cos(ang).astype(bf)
    sign = np.where(p % 2 == 0, -1.0, 1.0)[:, None]
    sin_t = (np.sin(ang) * sign).astype(bf)

    msw = np.zeros((128, 128), dtype=bf)
    msw[p, p ^ 1] = 1.0

    maps = []
    for core in range(N_CORES):
        b, hg = divmod(core, HG)
        hs = [HPG * hg + i for i in range(HPG)]
        w_qk = np.concatenate(
            [W_qkv[:, h * D : (h + 1) * D] for h in hs]
            + [W_qkv[:, ATT + h * D : ATT + (h + 1) * D] for h in hs],
            axis=1,
        )
        w_v = np.concatenate(
            [W_qkv[:, 2 * ATT + h * D : 2 * ATT + (h + 1) * D] for h in hs], axis=1
        )
        w_o = np.concatenate([W_out[h * D : (h + 1) * D, :] for h in hs], axis=0)
        xT = np.ascontiguousarray(x[b].T)
        m = {
            "w_qk": _pack_e(w_qk).astype(bf),
            "w_v": _pack_e(w_v).astype(bf),
            "w_o": np.ascontiguousarray(w_o).astype(bf),
            "cos_t": cos_t,
            "sin_t": sin_t,
            "mswap": msw,
        }
        for c in range(NSC):
            m[f"xT{c}"] = _pack_e(xT[:, 512 * c : 512 * (c + 1)]).astype(bf)
        maps.append(m)
    return maps


def _gather(res, inputs=None):
    out = np.zeros((B, S, E), dtype=np.float32)
    for core in range(N_CORES):
        b = core // HG
        out[b] += res.results[core]["out"]
    return out


def kernel(x, W_qkv, W_out):
    from concourse.bass_utils import run_bass_kernel_spmd

    nc = _get_program()
    maps = _host_inputs(x, W_qkv, W_out)
    res = run_bass_kernel_spmd(nc, maps, core_ids=list(range(N_CORES)))
    return _gather(res)


# revision 25
# speedup vs baseline: 1.1876x; 1.0173x over previous
"""Trainium2 Bass kernel for MultiHeadSelfAttention with RoPE (bf16 path).

Problem: x[2, 2048, 1024] @ W_qkv[1024, 3072] -> rope(q,k) -> softmax(q k^T/8) v
         -> out @ W_out[1024, 1024].

Sharding (8 cores): batch (2-way) x head-group (4-way, 4 heads each).
Each core computes a partial output [2048, 1024] = attnout_heads @ W_out_rows;
host sums the 4 head-group partials per batch.

All matmuls run in bf16 (inputs pre-cast + DMA-layout-packed on host so every
transfer moves >=2KB lines), accumulating in fp32 PSUM; elementwise work is
bf16 (DVE 2x packed mode).

Schedule: the attention inner loop is paced by ScalarE's exp (~1.2us per
[128,1024] pair-tile vs ~0.86us of PE work per sk tile), so all remaining PE
work is drained INTO those gaps as 1-2 matmul micro-steps per sk:
  qk-proj(pair0) ; v-proj  ->  attention units (ch,pair0) x4 with
  qk-proj(pair1) steps as background  ->  units (ch,pair1) x4 with the
  output projection of finished chunks as background  ->  tail outproj.

On-core dataflow is fully "transposed" so the PE never needs a transpose:
  qT,kT[c, s] = sum_e W[e, c] * xT[e, s]   (lhsT = W slice, rhs = xT)
  rot = Mswap @ qT (PE), q' = qT*cos + rot*sin_signed (DVE)
  scores[sk, sq] per head via K=128 packing: [kT_A|kT_B] against zero-padded
  q ([q_A|0] / [0|q_B]); both heads' 512-col scores land in one [128, 1024]
  PSUM tile so a single ScalarE exp (scale=1/8 folded) serves the pair.
  attnT[sk, sq] -> oT[d, sq] += [v|1]^T attn (ones column gives the softmax
  denominator in row 64 for free); normalize via ones-outer-product broadcast
  + reciprocal + multiply; out_partial[s, e] = att_oT.T @ W_out_rows.
"""

import sys

if "/opt/trn_rl_repo" not in sys.path:
    sys.path.insert(0, "/opt/trn_rl_repo")

import numpy as np

B, S, E = 2, 2048, 1024
ATT = 1024
H = 16
D = 64
HG = 4            # head groups (cores per batch)
HPG = H // HG     # heads per core = 4
PAIRS = HPG // 2  # head pairs per core = 2
ROPE_THETA = 10000.0
N_CORES = 8

CH = 512              # sq chunk for the attention inner loop
N_CH = S // CH        # 4 chunks
N_SK = S // 128       # 16 sk tiles
EK = E // 128         # 8 contraction tiles over embedding dim
NSC = S // 512        # 4 s-chunks for xT staging
WQCOLS = 2 * HPG * D  # 512 qk-weight columns per core

_BUILT = {}


def _build_program():
    import concourse.bacc as bacc
    import concourse.tile as tile
    import concourse.mybir as mybir

    f32 = mybir.dt.float32
    bf16 = mybir.dt.bfloat16
    AF = mybir.ActivationFunctionType

    nc = bacc.Bacc(
        "TRN2",
        target_bir_lowering=False,
        debug=False,
        enable_asserts=False,
        num_devices=N_CORES,
    )

    # Wide operands host-packed to [128, ...] with the 128-row contraction
    # tiles folded into the free dim, so each DMA moves contiguous >=2KB
    # lines per partition. w_qkm = 4 col-blocks of [e, 128] + mswap tail.
    xTs = [
        nc.dram_tensor(f"xT{c}", [128, EK * 512], bf16, kind="ExternalInput").ap()
        for c in range(NSC)
    ]
    w_qkm = nc.dram_tensor(
        "w_qkm", [128, 4 * EK * 128 + 128], bf16, kind="ExternalInput"
    ).ap()
    w_v = nc.dram_tensor("w_v", [128, EK * HPG * D], bf16, kind="ExternalInput").ap()
    w_o = nc.dram_tensor("w_o", [HPG * D, E], bf16, kind="ExternalInput").ap()
    cos_t = nc.dram_tensor("cos_t", [128, S], bf16, kind="ExternalInput").ap()
    sin_t = nc.dram_tensor("sin_t", [128, S], bf16, kind="ExternalInput").ap()
    out = nc.dram_tensor("out", [S, E], f32, kind="ExternalOutput").ap()

    with tile.TileContext(nc) as tc:
        with (
            tc.tile_pool(name="const", bufs=1) as constp,
            tc.tile_pool(name="qkT", bufs=1) as qkTp,
            tc.tile_pool(name="vsb", bufs=1) as vp,
            tc.tile_pool(name="attnout", bufs=1) as aop,
            tc.tile_pool(name="wo", bufs=1) as wop,
            tc.tile_pool(name="xt", bufs=NSC) as xtp,
            tc.tile_pool(name="wqk", bufs=1) as wqkp,
            tc.tile_pool(name="wv", bufs=1) as wvp,
            tc.tile_pool(name="ropes", bufs=3) as ropep,
            tc.tile_pool(name="trig", bufs=1) as trigp,
            tc.tile_pool(name="bgps", bufs=2, space="PSUM") as bgps,
            tc.tile_pool(name="attps", bufs=2, space="PSUM") as attps,
            tc.tile_pool(name="oTps", bufs=1, space="PSUM") as oTps,
            tc.tile_pool(name="expp", bufs=4) as expp,
            tc.tile_pool(name="recipp", bufs=2) as rcp,
            tc.tile_pool(name="osb", bufs=3) as osbp,
        ):
            # onesrow at partition 64 matches the denominator rhs operand
            onesrow = constp.tile([65, 64], bf16, tag="onesrow")
            nc.gpsimd.memset(onesrow[64:65, :], 1.0)
            # ACT warmup: exp table-set load off the critical path
            warm = constp.tile([65, 16], bf16, tag="warm")
            nc.scalar.activation(warm[64:65, :], onesrow[64:65, 0:16], AF.Exp, scale=0.125)

            qzlo = [qkTp.tile([128, S], bf16, tag=f"qzlo{g}", name=f"qzlo{g}") for g in range(PAIRS)]
            qzhi = [qkTp.tile([128, S], bf16, tag=f"qzhi{g}", name=f"qzhi{g}") for g in range(PAIRS)]
            kT = [qkTp.tile([128, S], bf16, tag=f"kT{g}", name=f"kT{g}") for g in range(PAIRS)]
            for g in range(PAIRS):
                nc.gpsimd.memset(qzlo[g][64:128, :], 0.0)
                nc.gpsimd.memset(qzhi[g][0:64, :], 0.0)
            # v natural + aug ones column, 4 heads: head h occupies cols
            # [65h, 65h+64) = v, col 65h+64 = ones (softmax-denominator row)
            v_c = vp.tile([128, N_SK, 4 * 65], bf16, tag="vc", name="vc")
            for h in range(4):
                nc.gpsimd.memset(v_c[:, :, 65 * h + 64], 1.0)
            att_o = [aop.tile([128, S], bf16, tag=f"ao{g}", name=f"ao{g}") for g in range(PAIRS)]
            wo_sb = [wop.tile([128, E], bf16, tag=f"wo{g}", name=f"wo{g}") for g in range(PAIRS)]

            cos_sb = trigp.tile([128, S], bf16, tag="cos")
            sin_sb = trigp.tile([128, S], bf16, tag="sin")
            # DMA rings: x chunks on sync, weights/trig on scalar.
            # wqkm[:, b, e, :] = W_qk cols [128b, 128b+128) for e-tile e;
            # wqkm[:, 4, 0, :] = mswap (rides the same 2KB-line transfer).
            wqkm = wqkp.tile([128, 4 * EK + 1, 128], bf16, tag="wqk")
            wqk_r = w_qkm.rearrange("p (b c) -> p b c", c=128)
            nc.scalar.dma_start(wqkm[:, 0:EK, :], wqk_r[:, 0:EK, :])
            nc.scalar.dma_start(cos_sb[:], cos_t[:])
            nc.scalar.dma_start(sin_sb[:], sin_t[:])
            nc.scalar.dma_start(wqkm[:, EK : 4 * EK + 1, :], wqk_r[:, EK : 4 * EK + 1, :])
            xt_c = []
            for c in range(NSC):
                t = xtp.tile([128, EK, 512], bf16, tag="xt")
                nc.sync.dma_start(t[:], xTs[c].rearrange("p (e s) -> p e s", e=EK))
                xt_c.append(t)
            wv_c = wvp.tile([128, EK, HPG * D], bf16, tag="wv")
            nc.scalar.dma_start(wv_c[:], w_v.rearrange("p (e c) -> p e c", e=EK))
            for g in range(PAIRS):
                nc.scalar.dma_start(wo_sb[g][:], w_o[128 * g : 128 * (g + 1), :])

            def wqk_ap(b, e):
                return wqkm[:, b * EK + e, :]

            msw_sb = wqkm[:, 4 * EK, :]

            # ---------------- micro-step machinery ----------------
            # Background PE work is emitted as single-matmul steps so it
            # drains into the ~0.3us/sk slack of the exp-paced inner loop.
            rope_pend = []

            def rope_tail():
                (g_, dest, sl, raw, ptag) = rope_pend.pop(0)
                rp = (bgps if ptag == "bg" else attps).tile([128, 512], f32, tag=ptag)
                nc.tensor.matmul(rp[:], msw_sb, raw[:], start=True, stop=True)
                rps = ropep.tile([128, 512], bf16, tag="rps")
                nc.scalar.copy(rps[:], rp[:])
                t2 = ropep.tile([128, 512], bf16, tag="t2")
                nc.vector.tensor_mul(t2[:], raw[:], cos_sb[:, sl])
                t1 = ropep.tile([128, 512], bf16, tag="t1")
                nc.vector.tensor_mul(t1[:], rps[:], sin_sb[:, sl])
                if dest is None:
                    nc.vector.tensor_add(qzlo[g_][0:64, sl], t1[0:64, :], t2[0:64, :])
                    nc.vector.tensor_add(qzhi[g_][64:128, sl], t1[64:128, :], t2[64:128, :])
                else:
                    nc.vector.tensor_add(dest[:, sl], t1[:], t2[:])

            def chain_steps(g, ti, c, ptag="bg"):
                """qk projection chain as EK single-matmul steps."""
                dest = None if ti == 0 else kT[g]
                b = 2 * ti + g
                sl = slice(512 * c, 512 * (c + 1))
                state = {}

                def mk(e):
                    def step():
                        if e == 0:
                            state["pp"] = (bgps if ptag == "bg" else attps).tile(
                                [128, 512], f32, tag=ptag, name=f"pp{g}{ti}{c}"
                            )
                        nc.tensor.matmul(
                            state["pp"][:],
                            wqk_ap(b, e),
                            xt_c[c][:, e, :],
                            start=(e == 0),
                            stop=(e == EK - 1),
                        )
                        if e == EK - 1:
                            raw = ropep.tile([128, 512], bf16, tag="raw")
                            nc.scalar.copy(raw[:], state["pp"][:])
                            rope_pend.append((g, dest, sl, raw, ptag))
                            if len(rope_pend) > 1:
                                rope_tail()

                    return step

                return [mk(e) for e in range(EK)]

            def proj_v(st):
                vp_ps = bgps.tile([128, 2 * 128], f32, tag="bg")
                for e in range(EK):
                    nc.tensor.matmul(
                        vp_ps[:],
                        xt_c[st // 4][:, e, 128 * (st % 4) : 128 * (st % 4 + 1)],
                        wv_c[:, e, :],
                        start=(e == 0),
                        stop=(e == EK - 1),
                    )
                nc.vector.tensor_copy(
                    v_c[:, st, 0 : 4 * 65].rearrange("p (h d) -> p h d", h=4)[:, :, 0:64],
                    vp_ps[:].rearrange("p (h d) -> p h d", h=4),
                )

            def outproj_steps(st):
                """output projection of one s-tile as two 2-matmul steps
                through the 1-bank bg slots + a DMA step."""
                ssl = slice(128 * st, 128 * (st + 1))
                state = {}

                def half(n):
                    def step():
                        if n == 0:
                            state["ot"] = osbp.tile(
                                [128, E], f32, tag="ot", name=f"ot{st}"
                            )
                        nsl = slice(512 * n, 512 * (n + 1))
                        op = bgps.tile([128, 512], f32, tag="bg")
                        for g in range(PAIRS):
                            nc.tensor.matmul(
                                op[:],
                                att_o[g][:, ssl],
                                wo_sb[g][:, nsl],
                                start=(g == 0),
                                stop=(g == PAIRS - 1),
                            )
                        nc.vector.tensor_copy(state["ot"][:, nsl], op[:])
                        if n == 1:
                            nc.sync.dma_start(out[ssl, :], state["ot"][:])

                    return step

                return [half(0), half(1)]

            def attention_unit(g, ch, bg_queue, bg_budget):
                """One (pair, chunk) unit; drains bg_budget steps from
                bg_queue across its 16 exp-paced sk iterations."""
                cslice = slice(CH * ch, CH * (ch + 1))
                hA, hB = 2 * g, 2 * g + 1
                oTA = oTps.tile([65, CH], f32, tag="oTA")
                oTB = oTps.tile([65, CH], f32, tag="oTB")
                exps = []

                def attnv(sk):
                    eAB = exps[sk]
                    nc.tensor.matmul(
                        oTA[:],
                        v_c[:, sk, 65 * hA : 65 * hA + 65],
                        eAB[:, 0:512],
                        start=(sk == 0),
                        stop=(sk == N_SK - 1),
                    )
                    nc.tensor.matmul(
                        oTB[:],
                        v_c[:, sk, 65 * hB : 65 * hB + 65],
                        eAB[:, 512:1024],
                        start=(sk == 0),
                        stop=(sk == N_SK - 1),
                    )

                drained = 0
                for sk in range(N_SK):
                    sksl = slice(128 * sk, 128 * (sk + 1))
                    sAB = attps.tile([128, 1024], f32, tag="sAB")
                    nc.tensor.matmul(
                        sAB[:, 0:512], kT[g][:, sksl], qzlo[g][:, cslice],
                        start=True, stop=True,
                    )
                    nc.tensor.matmul(
                        sAB[:, 512:1024], kT[g][:, sksl], qzhi[g][:, cslice],
                        start=True, stop=True,
                    )
                    eAB = expp.tile([128, 1024], bf16, tag="eAB")
                    nc.scalar.activation(eAB[:], sAB[:], AF.Exp, scale=0.125)
                    exps.append(eAB)
                    if sk > 0:
                        attnv(sk - 1)
                    want = (sk + 1) * bg_budget // N_SK
                    while drained < want and bg_queue:
                        bg_queue.pop(0)()
                        drained += 1
                attnv(N_SK - 1)

                # normalize: denominators live in row 64 of oTA/oTB.
                # Denom-row staging on ScalarE (idle at boundaries), db in
                # the bg slot, head-interleaved — the next unit's scores and
                # attnv never wait on this chain.
                oXA = rcp.tile([65, CH], bf16, tag="oX0")
                oXB = rcp.tile([65, CH], bf16, tag="oX1")
                nc.vector.tensor_copy(oXA[64:65, :], oTA[64:65, :])
                nc.vector.tensor_copy(oXB[64:65, :], oTB[64:65, :])
                dbA = bgps.tile([64, CH], f32, tag="bg")
                dbB = bgps.tile([64, CH], f32, tag="bg")
                nc.tensor.matmul(
                    dbA[:], onesrow[64:65, :], oXA[64:65, :], start=True, stop=True
                )
                nc.tensor.matmul(
                    dbB[:], onesrow[64:65, :], oXB[64:65, :], start=True, stop=True
                )
                rbA = rcp.tile([64, CH], f32, tag="rb0")
                rbB = rcp.tile([64, CH], f32, tag="rb1")
                nc.vector.reciprocal_approx_fast(rbA[:], dbA[:])
                nc.vector.reciprocal_approx_fast(rbB[:], dbB[:])
                nc.vector.tensor_mul(att_o[g][0:64, cslice], oTA[0:64, :], rbA[:])
                aoB = rcp.tile([64, CH], bf16, tag="aoB")
                nc.vector.tensor_mul(aoB[:], oTB[0:64, :], rbB[:])
                nc.sync.dma_start(att_o[g][64:128, cslice], aoB[:])

            # ---------------- emission ----------------
            # serial head: pair-0 projection (c-major so chains track the
            # xT chunk arrivals; chains alternate PSUM slots between the bg
            # tag and the still-idle scores tag) + all of v
            for i, (c, ti) in enumerate([(c, t) for c in range(NSC) for t in range(2)]):
                for step in chain_steps(0, ti, c, ptag=("bg", "sAB")[i % 2]):
                    step()
            while rope_pend:
                rope_tail()
            for st in range(N_SK):
                proj_v(st)

            # pair-0 attention with pair-1 projection as background
            g1_steps = []
            for c in range(NSC):
                for ti in range(2):
                    g1_steps.extend(chain_steps(1, ti, c))
            for ch in range(N_CH):
                attention_unit(0, ch, g1_steps, (len(g1_steps) + N_CH - 1 - ch) // (N_CH - ch))
            while g1_steps:
                g1_steps.pop(0)()
            while rope_pend:
                rope_tail()

            # pair-1 attention with finished chunks' output projection as
            # background (chunk ch-1 is complete once unit (1, ch-1) done)
            for ch in range(N_CH):
                op_steps = []
                if ch > 0:
                    for st in range(CH * (ch - 1) // 128, CH * ch // 128):
                        op_steps.extend(outproj_steps(st))
                attention_unit(1, ch, op_steps, len(op_steps))
                while op_steps:
                    op_steps.pop(0)()
            for st in range(CH * (N_CH - 1) // 128, S // 128):
                for step in outproj_steps(st):
                    step()

    nc.compile()
    return nc


def _get_program():
    if "nc" not in _BUILT:
        _BUILT["nc"] = _build_program()
    return _BUILT["nc"]


def _pack_e(a):
    """[E, C] -> [128, EK*C] with row p = concat over e of a[128e+p, :]."""
    Edim, C = a.shape
    return np.ascontiguousarray(
        a.reshape(EK, 128, C).transpose(1, 0, 2).reshape(128, EK * C)
    )


def _host_inputs(x, W_qkv, W_out):
    """Build the 8 per-core input maps (bf16, DMA-packed)."""
    import ml_dtypes

    bf = ml_dtypes.bfloat16
    f = np.float32
    x = np.asarray(x, dtype=f)
    W_qkv = np.asarray(W_qkv, dtype=f)
    W_out = np.asarray(W_out, dtype=f)

    inv_freq = 1.0 / (ROPE_THETA ** (np.arange(0, D, 2, dtype=np.float64) / D))
    p = np.arange(128)
    freq_row = inv_freq[(p % D) // 2]  # [128]
    ang = freq_row[:, None] * np.arange(S, dtype=np.float64)[None, :]  # [128, S]
    cos_t = np.cos(ang).astype(bf)
    sign = np.where(p % 2 == 0, -1.0, 1.0)[:, None]
    sin_t = (np.sin(ang) * sign).astype(bf)

    msw = np.zeros((128, 128), dtype=f)
    msw[p, p ^ 1] = 1.0

    maps = []
    for core in range(N_CORES):
        b, hg = divmod(core, HG)
        hs = [HPG * hg + i for i in range(HPG)]
        w_qk = np.concatenate(
            [W_qkv[:, h * D : (h + 1) * D] for h in hs]
            + [W_qkv[:, ATT + h * D : ATT + (h + 1) * D] for h in hs],
            axis=1,
        )
        w_v = np.concatenate(
            [W_qkv[:, 2 * ATT + h * D : 2 * ATT + (h + 1) * D] for h in hs], axis=1
        )
        w_o = np.concatenate([W_out[h * D : (h + 1) * D, :] for h in hs], axis=0)
        # wqkm: 4 col-blocks of [128, EK*128] + mswap appended
        blocks = [
            _pack_e(np.ascontiguousarray(w_qk[:, 128 * bb : 128 * (bb + 1)]))
            for bb in range(4)
        ]
        w_qkm = np.concatenate(blocks + [msw], axis=1)
        xT = np.ascontiguousarray(x[b].T)
        m = {
            "w_qkm": w_qkm.astype(bf),
            "w_v": _pack_e(w_v).astype(bf),
            "w_o": np.ascontiguousarray(w_o).astype(bf),
            "cos_t": cos_t,
            "sin_t": sin_t,
        }
        for c in range(NSC):
            m[f"xT{c}"] = _pack_e(xT[:, 512 * c : 512 * (c + 1)]).astype(bf)
        maps.append(m)
    return maps


def _gather(res, inputs=None):
    out = np.zeros((B, S, E), dtype=np.float32)
    for core in range(N_CORES):
        b = core // HG
        out[b] += res.results[core]["out"]
    return out


def kernel(x, W_qkv, W_out):
    from concourse.bass_utils import run_bass_kernel_spmd

    nc = _get_program()
    maps = _host_inputs(x, W_qkv, W_out)
    res = run_bass_kernel_spmd(nc, maps, core_ids=list(range(N_CORES)))
    return _gather(res)


# revision 28
# speedup vs baseline: 1.2456x; 1.0488x over previous
"""Trainium2 Bass kernel for MultiHeadSelfAttention with RoPE (bf16 path).

Problem: x[2, 2048, 1024] @ W_qkv[1024, 3072] -> rope(q,k) -> softmax(q k^T/8) v
         -> out @ W_out[1024, 1024].

Sharding (8 cores): batch (2-way) x head-group (4-way, 4 heads each).
Each core computes a partial output [2048, 1024] = attnout_heads @ W_out_rows;
host sums the 4 head-group partials per batch.

All matmuls run in bf16 (inputs pre-cast + DMA-layout-packed on host so every
transfer moves >=2KB lines), accumulating in fp32 PSUM; elementwise work is
bf16 (DVE 2x packed mode).

Schedule: the attention inner loop is paced by ScalarE's exp (~1.2us per
[128,1024] pair-tile vs ~0.86us of PE work per sk tile), so all remaining PE
work is drained INTO those gaps as 1-2 matmul micro-steps per sk:
  qk-proj(pair0) ; v-proj  ->  attention units (ch,pair0) x4 with
  qk-proj(pair1) steps as background  ->  units (ch,pair1) x4 with the
  output projection of finished chunks as background  ->  tail outproj.

On-core dataflow is fully "transposed" so the PE never needs a transpose:
  qT,kT[c, s] = sum_e W[e, c] * xT[e, s]   (lhsT = W slice, rhs = xT)
  rot = Mswap @ qT (PE), q' = qT*cos + rot*sin_signed (DVE)
  scores[sk, sq] per head via K=128 packing: [kT_A|kT_B] against zero-padded
  q ([q_A|0] / [0|q_B]); both heads' 512-col scores land in one [128, 1024]
  PSUM tile so a single ScalarE exp (scale=1/8 folded) serves the pair.
  attnT[sk, sq] -> oT[d, sq] += [v|1]^T attn (ones column gives the softmax
  denominator in row 64 for free); normalize via ones-outer-product broadcast
  + reciprocal + multiply; out_partial[s, e] = att_oT.T @ W_out_rows.
"""

import sys

if "/opt/trn_rl_repo" not in sys.path:
    sys.path.insert(0, "/opt/trn_rl_repo")

import numpy as np

B, S, E = 2, 2048, 1024
ATT = 1024
H = 16
D = 64
HG = 4            # head groups (cores per batch)
HPG = H // HG     # heads per core = 4
PAIRS = HPG // 2  # head pairs per core = 2
ROPE_THETA = 10000.0
N_CORES = 8

CH = 512              # sq chunk for the attention inner loop
N_CH = S // CH        # 4 chunks
N_SK = S // 128       # 16 sk tiles
EK = E // 128         # 8 contraction tiles over embedding dim
NSC = S // 512        # 4 s-chunks for xT staging
WQCOLS = 2 * HPG * D  # 512 qk-weight columns per core

_BUILT = {}


def _build_program():
    import concourse.bacc as bacc
    import concourse.tile as tile
    import concourse.mybir as mybir

    f32 = mybir.dt.float32
    bf16 = mybir.dt.bfloat16
    AF = mybir.ActivationFunctionType

    nc = bacc.Bacc(
        "TRN2",
        target_bir_lowering=False,
        debug=False,
        enable_asserts=False,
        num_devices=N_CORES,
    )

    # Wide operands host-packed to [128, ...] with the 128-row contraction
    # tiles folded into the free dim, so each DMA moves contiguous >=2KB
    # lines per partition. w_qkm = 4 col-blocks of [e, 128] + mswap tail.
    xTs = [
        nc.dram_tensor(f"xT{c}", [128, EK * 512], bf16, kind="ExternalInput").ap()
        for c in range(NSC)
    ]
    w_qkm = nc.dram_tensor(
        "w_qkm", [128, 4 * EK * 128 + 128], bf16, kind="ExternalInput"
    ).ap()
    w_v = nc.dram_tensor("w_v", [128, EK * HPG * D], bf16, kind="ExternalInput").ap()
    w_o = nc.dram_tensor("w_o", [HPG * D, E], bf16, kind="ExternalInput").ap()
    cos_t = nc.dram_tensor("cos_t", [128, S], bf16, kind="ExternalInput").ap()
    sin_t = nc.dram_tensor("sin_t", [128, S], bf16, kind="ExternalInput").ap()
    out = nc.dram_tensor("out", [S, E], f32, kind="ExternalOutput").ap()

    with tile.TileContext(nc) as tc:
        with (
            tc.tile_pool(name="const", bufs=1) as constp,
            tc.tile_pool(name="qkT", bufs=1) as qkTp,
            tc.tile_pool(name="vsb", bufs=1) as vp,
            tc.tile_pool(name="attnout", bufs=1) as aop,
            tc.tile_pool(name="wo", bufs=1) as wop,
            tc.tile_pool(name="xt", bufs=NSC) as xtp,
            tc.tile_pool(name="wqk", bufs=1) as wqkp,
            tc.tile_pool(name="wv", bufs=1) as wvp,
            tc.tile_pool(name="ropes", bufs=3) as ropep,
            tc.tile_pool(name="trig", bufs=1) as trigp,
            tc.tile_pool(name="bgps", bufs=2, space="PSUM") as bgps,
            tc.tile_pool(name="attps", bufs=2, space="PSUM") as attps,
            tc.tile_pool(name="oTps", bufs=1, space="PSUM") as oTps,
            tc.tile_pool(name="expp", bufs=4) as expp,
            tc.tile_pool(name="recipp", bufs=2) as rcp,
            tc.tile_pool(name="osb", bufs=3) as osbp,
        ):
            # onesrow at partition 64 matches the denominator rhs operand
            onesrow = constp.tile([65, 64], bf16, tag="onesrow")
            nc.gpsimd.memset(onesrow[64:65, :], 1.0)
            # ACT warmup: exp table-set load off the critical path
            warm = constp.tile([65, 16], bf16, tag="warm")
            nc.scalar.activation(warm[64:65, :], onesrow[64:65, 0:16], AF.Exp, scale=0.125)

            qzlo = [qkTp.tile([128, S], bf16, tag=f"qzlo{g}", name=f"qzlo{g}") for g in range(PAIRS)]
            qzhi = [qkTp.tile([128, S], bf16, tag=f"qzhi{g}", name=f"qzhi{g}") for g in range(PAIRS)]
            kT = [qkTp.tile([128, S], bf16, tag=f"kT{g}", name=f"kT{g}") for g in range(PAIRS)]
            for g in range(PAIRS):
                nc.gpsimd.memset(qzlo[g][64:128, :], 0.0)
                nc.gpsimd.memset(qzhi[g][0:64, :], 0.0)
            # v natural + aug ones column, 4 heads: head h occupies cols
            # [65h, 65h+64) = v, col 65h+64 = ones (softmax-denominator row)
            v_c = vp.tile([128, N_SK, 4 * 65], bf16, tag="vc", name="vc")
            for h in range(4):
                nc.gpsimd.memset(v_c[:, :, 65 * h + 64], 1.0)
            att_o = [aop.tile([128, S], bf16, tag=f"ao{g}", name=f"ao{g}") for g in range(PAIRS)]
            wo_sb = [wop.tile([128, E], bf16, tag=f"wo{g}", name=f"wo{g}") for g in range(PAIRS)]

            cos_sb = trigp.tile([128, S], bf16, tag="cos")
            sin_sb = trigp.tile([128, S], bf16, tag="sin")
            # DMA rings: x chunks on sync, weights/trig on scalar.
            # wqkm[:, b, e, :] = W_qk cols [128b, 128b+128) for e-tile e;
            # wqkm[:, 4, 0, :] = mswap (rides the same 2KB-line transfer).
            wqkm = wqkp.tile([128, 4 * EK + 1, 128], bf16, tag="wqk")
            wqk_r = w_qkm.rearrange("p (b c) -> p b c", c=128)
            nc.scalar.dma_start(wqkm[:, 0:EK, :], wqk_r[:, 0:EK, :])
            nc.scalar.dma_start(wqkm[:, EK : 4 * EK + 1, :], wqk_r[:, EK : 4 * EK + 1, :])
            nc.scalar.dma_start(cos_sb[:], cos_t[:])
            nc.scalar.dma_start(sin_sb[:], sin_t[:])
            xt_c = []
            for c in range(NSC):
                t = xtp.tile([128, EK, 512], bf16, tag="xt")
                nc.sync.dma_start(t[:], xTs[c].rearrange("p (e s) -> p e s", e=EK))
                xt_c.append(t)
            wv_c = wvp.tile([128, EK, HPG * D], bf16, tag="wv")
            nc.scalar.dma_start(wv_c[:], w_v.rearrange("p (e c) -> p e c", e=EK))
            for g in range(PAIRS):
                nc.scalar.dma_start(wo_sb[g][:], w_o[128 * g : 128 * (g + 1), :])

            def wqk_ap(b, e):
                return wqkm[:, b * EK + e, :]

            msw_sb = wqkm[:, 4 * EK, :]

            # ---------------- micro-step machinery ----------------
            # Background PE work is emitted as single-matmul steps so it
            # drains into the ~0.3us/sk slack of the exp-paced inner loop.
            rope_pend = []

            def rope_tail():
                (g_, dest, sl, raw, ptag) = rope_pend.pop(0)
                rp = (bgps if ptag == "bg" else attps).tile([128, 512], f32, tag=ptag)
                nc.tensor.matmul(rp[:], msw_sb, raw[:], start=True, stop=True)
                rps = ropep.tile([128, 512], bf16, tag="rps")
                nc.scalar.copy(rps[:], rp[:])
                t2 = ropep.tile([128, 512], bf16, tag="t2")
                nc.vector.tensor_mul(t2[:], raw[:], cos_sb[:, sl])
                t1 = ropep.tile([128, 512], bf16, tag="t1")
                nc.vector.tensor_mul(t1[:], rps[:], sin_sb[:, sl])
                if dest is None:
                    nc.vector.tensor_add(qzlo[g_][0:64, sl], t1[0:64, :], t2[0:64, :])
                    nc.vector.tensor_add(qzhi[g_][64:128, sl], t1[64:128, :], t2[64:128, :])
                else:
                    nc.vector.tensor_add(dest[:, sl], t1[:], t2[:])

            def chain_steps(g, ti, c, ptag="bg"):
                """qk projection chain as EK single-matmul steps."""
                dest = None if ti == 0 else kT[g]
                b = 2 * ti + g
                sl = slice(512 * c, 512 * (c + 1))
                state = {}

                def mk(e):
                    def step():
                        if e == 0:
                            state["pp"] = (bgps if ptag == "bg" else attps).tile(
                                [128, 512], f32, tag=ptag, name=f"pp{g}{ti}{c}"
                            )
                        nc.tensor.matmul(
                            state["pp"][:],
                            wqk_ap(b, e),
                            xt_c[c][:, e, :],
                            start=(e == 0),
                            stop=(e == EK - 1),
                        )
                        if e == EK - 1:
                            raw = ropep.tile([128, 512], bf16, tag="raw")
                            nc.scalar.copy(raw[:], state["pp"][:])
                            rope_pend.append((g, dest, sl, raw, ptag))
                            if len(rope_pend) > 1:
                                rope_tail()

                    return step

                return [mk(e) for e in range(EK)]

            def proj_v(st):
                vp_ps = bgps.tile([128, 2 * 128], f32, tag="bg")
                for e in range(EK):
                    nc.tensor.matmul(
                        vp_ps[:],
                        xt_c[st // 4][:, e, 128 * (st % 4) : 128 * (st % 4 + 1)],
                        wv_c[:, e, :],
                        start=(e == 0),
                        stop=(e == EK - 1),
                    )
                nc.vector.tensor_copy(
                    v_c[:, st, 0 : 4 * 65].rearrange("p (h d) -> p h d", h=4)[:, :, 0:64],
                    vp_ps[:].rearrange("p (h d) -> p h d", h=4),
                )

            def outproj_steps(st):
                """output projection of one s-tile as two 2-matmul steps
                through the 1-bank bg slots + a DMA step."""
                ssl = slice(128 * st, 128 * (st + 1))
                state = {}

                def half(n):
                    def step():
                        if n == 0:
                            state["ot"] = osbp.tile(
                                [128, E], f32, tag="ot", name=f"ot{st}"
                            )
                        nsl = slice(512 * n, 512 * (n + 1))
                        op = bgps.tile([128, 512], f32, tag="bg")
                        for g in range(PAIRS):
                            nc.tensor.matmul(
                                op[:],
                                att_o[g][:, ssl],
                                wo_sb[g][:, nsl],
                                start=(g == 0),
                                stop=(g == PAIRS - 1),
                            )
                        # alternate evacuation engine so back-to-back
                        # outproj steps in the tail don't serialize on DVE
                        if n == 0:
                            nc.vector.tensor_copy(state["ot"][:, nsl], op[:])
                        else:
                            nc.scalar.copy(state["ot"][:, nsl], op[:])
                            nc.sync.dma_start(out[ssl, :], state["ot"][:])

                    return step

                return [half(0), half(1)]

            def attention_unit(g, ch, bg_queue, bg_budget):
                """One (pair, chunk) unit; drains bg_budget steps from
                bg_queue across its 16 exp-paced sk iterations."""
                cslice = slice(CH * ch, CH * (ch + 1))
                hA, hB = 2 * g, 2 * g + 1
                oTA = oTps.tile([65, CH], f32, tag="oTA")
                oTB = oTps.tile([65, CH], f32, tag="oTB")
                exps = []

                def attnv(sk):
                    eAB = exps[sk]
                    nc.tensor.matmul(
                        oTA[:],
                        v_c[:, sk, 65 * hA : 65 * hA + 65],
                        eAB[:, 0:512],
                        start=(sk == 0),
                        stop=(sk == N_SK - 1),
                    )
                    nc.tensor.matmul(
                        oTB[:],
                        v_c[:, sk, 65 * hB : 65 * hB + 65],
                        eAB[:, 512:1024],
                        start=(sk == 0),
                        stop=(sk == N_SK - 1),
                    )

                drained = 0
                for sk in range(N_SK):
                    sksl = slice(128 * sk, 128 * (sk + 1))
                    sAB = attps.tile([128, 1024], f32, tag="sAB")
                    nc.tensor.matmul(
                        sAB[:, 0:512], kT[g][:, sksl], qzlo[g][:, cslice],
                        start=True, stop=True,
                    )
                    nc.tensor.matmul(
                        sAB[:, 512:1024], kT[g][:, sksl], qzhi[g][:, cslice],
                        start=True, stop=True,
                    )
                    eAB = expp.tile([128, 1024], bf16, tag="eAB")
                    nc.scalar.activation(eAB[:], sAB[:], AF.Exp, scale=0.125)
                    exps.append(eAB)
                    if sk > 0:
                        attnv(sk - 1)
                    want = (sk + 1) * bg_budget // N_SK
                    while drained < want and bg_queue:
                        bg_queue.pop(0)()
                        drained += 1
                attnv(N_SK - 1)

                # normalize: denominators live in row 64 of oTA/oTB.
                # Denom-row staging on ScalarE (idle at boundaries), db in
                # the bg slot, head-interleaved — the next unit's scores and
                # attnv never wait on this chain.
                oXA = rcp.tile([65, CH], bf16, tag="oX0")
                oXB = rcp.tile([65, CH], bf16, tag="oX1")
                nc.vector.tensor_copy(oXA[64:65, :], oTA[64:65, :])
                nc.vector.tensor_copy(oXB[64:65, :], oTB[64:65, :])
                dbA = bgps.tile([64, CH], f32, tag="bg")
                dbB = bgps.tile([64, CH], f32, tag="bg")
                nc.tensor.matmul(
                    dbA[:], onesrow[64:65, :], oXA[64:65, :], start=True, stop=True
                )
                nc.tensor.matmul(
                    dbB[:], onesrow[64:65, :], oXB[64:65, :], start=True, stop=True
                )
                rbA = rcp.tile([64, CH], f32, tag="rb0")
                rbB = rcp.tile([64, CH], f32, tag="rb1")
                nc.vector.reciprocal_approx_fast(rbA[:], dbA[:])
                nc.vector.reciprocal_approx_fast(rbB[:], dbB[:])
                nc.vector.tensor_mul(att_o[g][0:64, cslice], oTA[0:64, :], rbA[:])
                aoB = rcp.tile([64, CH], bf16, tag="aoB")
                nc.vector.tensor_mul(aoB[:], oTB[0:64, :], rbB[:])
                nc.sync.dma_start(att_o[g][64:128, cslice], aoB[:])

            # ---------------- emission ----------------
            # serial head: pair-0 projection, c-major so the chain pace
            # (~3.4us per xT chunk for q+k) matches the xT chunk DMA
            # arrivals; chains alternate PSUM slots between the bg tag and
            # the still-idle scores tag. Then all of v.
            for i, (c, ti) in enumerate([(c, t) for c in range(NSC) for t in range(2)]):
                for step in chain_steps(0, ti, c, ptag=("bg", "sAB")[i % 2]):
                    step()
            while rope_pend:
                rope_tail()
            for st in range(N_SK):
                proj_v(st)

            # pair-0 attention with pair-1 projection as background
            g1_steps = []
            for c in range(NSC):
                for ti in range(2):
                    g1_steps.extend(chain_steps(1, ti, c))
            for ch in range(N_CH):
                attention_unit(0, ch, g1_steps, (len(g1_steps) + N_CH - 1 - ch) // (N_CH - ch))
            while g1_steps:
                g1_steps.pop(0)()
            while rope_pend:
                rope_tail()

            # pair-1 attention with finished chunks' output projection as
            # background (chunk ch-1 is complete once unit (1, ch-1) done)
            for ch in range(N_CH):
                op_steps = []
                if ch > 0:
                    for st in range(CH * (ch - 1) // 128, CH * ch // 128):
                        op_steps.extend(outproj_steps(st))
                attention_unit(1, ch, op_steps, len(op_steps))
                while op_steps:
                    op_steps.pop(0)()
            for st in range(CH * (N_CH - 1) // 128, S // 128):
                for step in outproj_steps(st):
                    step()

    nc.compile()
    return nc


def _get_program():
    if "nc" not in _BUILT:
        _BUILT["nc"] = _build_program()
    return _BUILT["nc"]


def _pack_e(a):
    """[E, C] -> [128, EK*C] with row p = concat over e of a[128e+p, :]."""
    Edim, C = a.shape
    return np.ascontiguousarray(
        a.reshape(EK, 128, C).transpose(1, 0, 2).reshape(128, EK * C)
    )


def _host_inputs(x, W_qkv, W_out):
    """Build the 8 per-core input maps (bf16, DMA-packed)."""
    import ml_dtypes

    bf = ml_dtypes.bfloat16
    f = np.float32
    x = np.asarray(x, dtype=f)
    W_qkv = np.asarray(W_qkv, dtype=f)
    W_out = np.asarray(W_out, dtype=f)

    inv_freq = 1.0 / (ROPE_THETA ** (np.arange(0, D, 2, dtype=np.float64) / D))
    p = np.arange(128)
    freq_row = inv_freq[(p % D) // 2]  # [128]
    ang = freq_row[:, None] * np.arange(S, dtype=np.float64)[None, :]  # [128, S]
    cos_t = np.cos(ang).astype(bf)
    sign = np.where(p % 2 == 0, -1.0, 1.0)[:, None]
    sin_t = (np.sin(ang) * sign).astype(bf)

    msw = np.zeros((128, 128), dtype=f)
    msw[p, p ^ 1] = 1.0

    maps = []
    for core in range(N_CORES):
        b, hg = divmod(core, HG)
        hs = [HPG * hg + i for i in range(HPG)]
        w_qk = np.concatenate(
            [W_qkv[:, h * D : (h + 1) * D] for h in hs]
            + [W_qkv[:, ATT + h * D : ATT + (h + 1) * D] for h in hs],
            axis=1,
        )
        w_v = np.concatenate(
            [W_qkv[:, 2 * ATT + h * D : 2 * ATT + (h + 1) * D] for h in hs], axis=1
        )
        w_o = np.concatenate([W_out[h * D : (h + 1) * D, :] for h in hs], axis=0)
        # wqkm: 4 col-blocks of [128, EK*128] + mswap appended
        blocks = [
            _pack_e(np.ascontiguousarray(w_qk[:, 128 * bb : 128 * (bb + 1)]))
            for bb in range(4)
        ]
        w_qkm = np.concatenate(blocks + [msw], axis=1)
        xT = np.ascontiguousarray(x[b].T)
        m = {
            "w_qkm": w_qkm.astype(bf),
            "w_v": _pack_e(w_v).astype(bf),
            "w_o": np.ascontiguousarray(w_o).astype(bf),
            "cos_t": cos_t,
            "sin_t": sin_t,
        }
        for c in range(NSC):
            m[f"xT{c}"] = _pack_e(xT[:, 512 * c : 512 * (c + 1)]).astype(bf)
        maps.append(m)
    return maps


def _gather(res, inputs=None):
    out = np.zeros((B, S, E), dtype=np.float32)
    for core in range(N_CORES):
        b = core // HG
        out[b] += res.results[core]["out"]
    return out


def kernel(x, W_qkv, W_out):
    from concourse.bass_utils import run_bass_kernel_spmd

    nc = _get_program()
    maps = _host_inputs(x, W_qkv, W_out)
    res = run_bass_kernel_spmd(nc, maps, core_ids=list(range(N_CORES)))
    return _gather(res)


# revision 33
# speedup vs baseline: 1.2730x; 1.0220x over previous
"""Trainium2 Bass kernel for MultiHeadSelfAttention with RoPE (bf16 path).

Problem: x[2, 2048, 1024] @ W_qkv[1024, 3072] -> rope(q,k) -> softmax(q k^T/8) v
         -> out @ W_out[1024, 1024].

Sharding (8 cores): batch (2-way) x head-group (4-way, 4 heads each).
Each core computes a partial output [2048, 1024] = attnout_heads @ W_out_rows;
host sums the 4 head-group partials per batch.

All matmuls run in bf16 (inputs pre-cast + DMA-layout-packed on host so every
transfer moves >=2KB lines), accumulating in fp32 PSUM; elementwise work is
bf16 (DVE 2x packed mode).

Schedule: the attention inner loop is paced by ScalarE's exp (~1.2us per
[128,1024] pair-tile vs ~0.86us of PE work per sk tile), so all remaining PE
work is drained INTO those gaps as 1-2 matmul micro-steps per sk:
  qk-proj(pair0) ; v-proj  ->  attention units (ch,pair0) x4 with
  qk-proj(pair1) steps as background  ->  units (ch,pair1) x4 with the
  output projection of finished chunks as background  ->  tail outproj.

On-core dataflow is fully "transposed" so the PE never needs a transpose:
  qT,kT[c, s] = sum_e W[e, c] * xT[e, s]   (lhsT = W slice, rhs = xT)
  rot = Mswap @ qT (PE), q' = qT*cos + rot*sin_signed (DVE)
  scores[sk, sq] per head via K=128 packing: [kT_A|kT_B] against zero-padded
  q ([q_A|0] / [0|q_B]); both heads' 512-col scores land in one [128, 1024]
  PSUM tile so a single ScalarE exp (scale=1/8 folded) serves the pair.
  attnT[sk, sq] -> oT[d, sq] += [v|1]^T attn (ones column gives the softmax
  denominator in row 64 for free); normalize via ones-outer-product broadcast
  + reciprocal + multiply; out_partial[s, e] = att_oT.T @ W_out_rows.
"""

import sys

if "/opt/trn_rl_repo" not in sys.path:
    sys.path.insert(0, "/opt/trn_rl_repo")

import numpy as np

B, S, E = 2, 2048, 1024
ATT = 1024
H = 16
D = 64
HG = 4            # head groups (cores per batch)
HPG = H // HG     # heads per core = 4
PAIRS = HPG // 2  # head pairs per core = 2
ROPE_THETA = 10000.0
N_CORES = 8

CH = 512              # sq chunk for the attention inner loop
N_CH = S // CH        # 4 chunks
N_SK = S // 128       # 16 sk tiles
EK = E // 128         # 8 contraction tiles over embedding dim
NSC = S // 512        # 4 s-chunks for xT staging
WQCOLS = 2 * HPG * D  # 512 qk-weight columns per core

_BUILT = {}


def _build_program():
    import concourse.bacc as bacc
    import concourse.tile as tile
    import concourse.mybir as mybir

    f32 = mybir.dt.float32
    bf16 = mybir.dt.bfloat16
    AF = mybir.ActivationFunctionType

    nc = bacc.Bacc(
        "TRN2",
        target_bir_lowering=False,
        debug=False,
        enable_asserts=False,
        num_devices=N_CORES,
    )

    # Wide operands host-packed to [128, ...] with the 128-row contraction
    # tiles folded into the free dim, so each DMA moves contiguous >=2KB
    # lines per partition. w_qkm = 4 col-blocks of [e, 128] + mswap tail.
    xTs = [
        nc.dram_tensor(f"xT{c}", [128, EK * 512], bf16, kind="ExternalInput").ap()
        for c in range(NSC)
    ]
    w_qkm = nc.dram_tensor(
        "w_qkm", [128, 4 * EK * 128 + 128], bf16, kind="ExternalInput"
    ).ap()
    w_v = nc.dram_tensor("w_v", [128, EK * HPG * D], bf16, kind="ExternalInput").ap()
    w_o = nc.dram_tensor("w_o", [HPG * D, E], bf16, kind="ExternalInput").ap()
    cos_t = nc.dram_tensor("cos_t", [128, S], bf16, kind="ExternalInput").ap()
    sin_t = nc.dram_tensor("sin_t", [128, S], bf16, kind="ExternalInput").ap()
    out = nc.dram_tensor("out", [S, E], f32, kind="ExternalOutput").ap()

    with tile.TileContext(nc) as tc:
        with (
            tc.tile_pool(name="const", bufs=1) as constp,
            tc.tile_pool(name="qkT", bufs=1) as qkTp,
            tc.tile_pool(name="vsb", bufs=1) as vp,
            tc.tile_pool(name="attnout", bufs=1) as aop,
            tc.tile_pool(name="wo", bufs=1) as wop,
            tc.tile_pool(name="xt", bufs=NSC) as xtp,
            tc.tile_pool(name="wqk", bufs=1) as wqkp,
            tc.tile_pool(name="wv", bufs=1) as wvp,
            tc.tile_pool(name="ropes", bufs=3) as ropep,
            tc.tile_pool(name="trig", bufs=1) as trigp,
            tc.tile_pool(name="bgps", bufs=2, space="PSUM") as bgps,
            tc.tile_pool(name="attps", bufs=2, space="PSUM") as attps,
            tc.tile_pool(name="oTps", bufs=1, space="PSUM") as oTps,
            tc.tile_pool(name="expp", bufs=6) as expp,
            tc.tile_pool(name="recipp", bufs=2) as rcp,
            tc.tile_pool(name="osb", bufs=3) as osbp,
        ):
            # onesrow at partition 64 matches the denominator rhs operand
            onesrow = constp.tile([65, 64], bf16, tag="onesrow")
            nc.gpsimd.memset(onesrow[64:65, :], 1.0)
            # ACT warmup: exp table-set load off the critical path
            warm = constp.tile([65, 16], bf16, tag="warm")
            nc.scalar.activation(warm[64:65, :], onesrow[64:65, 0:16], AF.Exp, scale=0.125)

            qzlo = [qkTp.tile([128, S], bf16, tag=f"qzlo{g}", name=f"qzlo{g}") for g in range(PAIRS)]
            qzhi = [qkTp.tile([128, S], bf16, tag=f"qzhi{g}", name=f"qzhi{g}") for g in range(PAIRS)]
            kT = [qkTp.tile([128, S], bf16, tag=f"kT{g}", name=f"kT{g}") for g in range(PAIRS)]
            for g in range(PAIRS):
                nc.gpsimd.memset(qzlo[g][64:128, :], 0.0)
                nc.gpsimd.memset(qzhi[g][0:64, :], 0.0)
            # v natural + aug ones column, 4 heads: head h occupies cols
            # [65h, 65h+64) = v, col 65h+64 = ones (softmax-denominator row)
            v_c = vp.tile([128, N_SK, 4 * 65], bf16, tag="vc", name="vc")
            for h in range(4):
                nc.gpsimd.memset(v_c[:, :, 65 * h + 64], 1.0)
            att_o = [aop.tile([128, S], bf16, tag=f"ao{g}", name=f"ao{g}") for g in range(PAIRS)]
            wo_sb = [wop.tile([128, E], bf16, tag=f"wo{g}", name=f"wo{g}") for g in range(PAIRS)]

            cos_sb = trigp.tile([128, S], bf16, tag="cos")
            sin_sb = trigp.tile([128, S], bf16, tag="sin")
            # DMA rings: x chunks on sync, weights/trig on scalar.
            # wqkm[:, b, e, :] = W_qk cols [128b, 128b+128) for e-tile e;
            # wqkm[:, 4, 0, :] = mswap (rides the same 2KB-line transfer).
            wqkm = wqkp.tile([128, 4 * EK + 1, 128], bf16, tag="wqk")
            wqk_r = w_qkm.rearrange("p (b c) -> p b c", c=128)
            nc.scalar.dma_start(wqkm[:, 0:EK, :], wqk_r[:, 0:EK, :])
            nc.scalar.dma_start(wqkm[:, EK : 4 * EK + 1, :], wqk_r[:, EK : 4 * EK + 1, :])
            nc.scalar.dma_start(cos_sb[:], cos_t[:])
            nc.scalar.dma_start(sin_sb[:], sin_t[:])
            xt_c = []
            for c in range(NSC):
                t = xtp.tile([128, EK, 512], bf16, tag="xt")
                nc.sync.dma_start(t[:], xTs[c].rearrange("p (e s) -> p e s", e=EK))
                xt_c.append(t)
            wv_c = wvp.tile([128, EK, HPG * D], bf16, tag="wv")
            nc.scalar.dma_start(wv_c[:], w_v.rearrange("p (e c) -> p e c", e=EK))
            for g in range(PAIRS):
                nc.scalar.dma_start(wo_sb[g][:], w_o[128 * g : 128 * (g + 1), :])

            def wqk_ap(b, e):
                return wqkm[:, b * EK + e, :]

            msw_sb = wqkm[:, 4 * EK, :]

            # ---------------- micro-step machinery ----------------
            # Background PE work is emitted as single-matmul steps so it
            # drains into the ~0.3us/sk slack of the exp-paced inner loop.
            rope_pend = []

            def rope_tail():
                (g_, dest, sl, raw, ptag, copy_eng) = rope_pend.pop(0)
                rp = (bgps if ptag == "bg" else attps).tile([128, 512], f32, tag=ptag)
                nc.tensor.matmul(rp[:], msw_sb, raw[:], start=True, stop=True)
                rps = ropep.tile([128, 512], bf16, tag="rps")
                if copy_eng == "scalar":
                    nc.scalar.copy(rps[:], rp[:])
                else:
                    nc.vector.tensor_copy(rps[:], rp[:])
                t2 = ropep.tile([128, 512], bf16, tag="t2")
                nc.vector.tensor_mul(t2[:], raw[:], cos_sb[:, sl])
                t1 = ropep.tile([128, 512], bf16, tag="t1")
                nc.vector.tensor_mul(t1[:], rps[:], sin_sb[:, sl])
                if dest is None:
                    nc.vector.tensor_add(qzlo[g_][0:64, sl], t1[0:64, :], t2[0:64, :])
                    nc.vector.tensor_add(qzhi[g_][64:128, sl], t1[64:128, :], t2[64:128, :])
                else:
                    nc.vector.tensor_add(dest[:, sl], t1[:], t2[:])

            def chain_steps(g, ti, c, ptag="bg", copy_eng=None):
                """qk projection chain as EK single-matmul steps. copy_eng
                does the PSUM evacuations: ScalarE in the serial head (idle
                there), VectorE for background chains inside attention (the
                exp stream must own ScalarE)."""
                dest = None if ti == 0 else kT[g]
                b = 2 * ti + g
                sl = slice(512 * c, 512 * (c + 1))
                state = {}

                def mk(e):
                    def step():
                        if e == 0:
                            state["pp"] = (bgps if ptag == "bg" else attps).tile(
                                [128, 512], f32, tag=ptag, name=f"pp{g}{ti}{c}"
                            )
                        nc.tensor.matmul(
                            state["pp"][:],
                            wqk_ap(b, e),
                            xt_c[c][:, e, :],
                            start=(e == 0),
                            stop=(e == EK - 1),
                        )
                        if e == EK - 1:
                            raw = ropep.tile([128, 512], bf16, tag="raw")
                            if copy_eng == "scalar":
                                nc.scalar.copy(raw[:], state["pp"][:])
                            else:
                                nc.vector.tensor_copy(raw[:], state["pp"][:])
                            rope_pend.append((g, dest, sl, raw, ptag, copy_eng))
                            if len(rope_pend) > 1:
                                rope_tail()

                    return step

                return [mk(e) for e in range(EK)]

            def proj_v(st):
                vp_ps = bgps.tile([128, 2 * 128], f32, tag="bg")
                for e in range(EK):
                    nc.tensor.matmul(
                        vp_ps[:],
                        xt_c[st // 4][:, e, 128 * (st % 4) : 128 * (st % 4 + 1)],
                        wv_c[:, e, :],
                        start=(e == 0),
                        stop=(e == EK - 1),
                    )
                nc.vector.tensor_copy(
                    v_c[:, st, 0 : 4 * 65].rearrange("p (h d) -> p h d", h=4)[:, :, 0:64],
                    vp_ps[:].rearrange("p (h d) -> p h d", h=4),
                )

            def outproj_steps(st, tail=False):
                """output projection of one s-tile as two 2-matmul steps
                through the 1-bank bg slots + a DMA step."""
                ssl = slice(128 * st, 128 * (st + 1))
                state = {}

                def half(n):
                    def step():
                        if n == 0:
                            state["ot"] = osbp.tile(
                                [128, E], f32, tag="ot", name=f"ot{st}"
                            )
                        nsl = slice(512 * n, 512 * (n + 1))
                        op = bgps.tile([128, 512], f32, tag="bg")
                        for g in range(PAIRS):
                            nc.tensor.matmul(
                                op[:],
                                att_o[g][:, ssl],
                                wo_sb[g][:, nsl],
                                start=(g == 0),
                                stop=(g == PAIRS - 1),
                            )
                        # in the tail ScalarE is idle (exp stream over):
                        # alternate engines; during attention keep ScalarE
                        # exclusively on exps
                        if tail and n == 1:
                            nc.scalar.copy(state["ot"][:, nsl], op[:])
                        else:
                            nc.vector.tensor_copy(state["ot"][:, nsl], op[:])
                        if n == 1:
                            nc.sync.dma_start(out[ssl, :], state["ot"][:])

                    return step

                return [half(0), half(1)]

            def attention_unit(g, ch, bg_queue, bg_budget):
                """One (pair, chunk) unit; drains bg_budget steps from
                bg_queue across its 16 exp-paced sk iterations."""
                cslice = slice(CH * ch, CH * (ch + 1))
                hA, hB = 2 * g, 2 * g + 1
                oTA = oTps.tile([65, CH], f32, tag="oTA")
                oTB = oTps.tile([65, CH], f32, tag="oTB")
                exps = []

                def attnv(sk):
                    eAB = exps[sk]
                    nc.tensor.matmul(
                        oTA[:],
                        v_c[:, sk, 65 * hA : 65 * hA + 65],
                        eAB[:, 0:512],
                        start=(sk == 0),
                        stop=(sk == N_SK - 1),
                    )
                    nc.tensor.matmul(
                        oTB[:],
                        v_c[:, sk, 65 * hB : 65 * hB + 65],
                        eAB[:, 512:1024],
                        start=(sk == 0),
                        stop=(sk == N_SK - 1),
                    )

                drained = 0
                for sk in range(N_SK):
                    sksl = slice(128 * sk, 128 * (sk + 1))
                    sAB = attps.tile([128, 1024], f32, tag="sAB")
                    nc.tensor.matmul(
                        sAB[:, 0:512], kT[g][:, sksl], qzlo[g][:, cslice],
                        start=True, stop=True,
                    )
                    nc.tensor.matmul(
                        sAB[:, 512:1024], kT[g][:, sksl], qzhi[g][:, cslice],
                        start=True, stop=True,
                    )
                    eAB = expp.tile([128, 1024], bf16, tag="eAB")
                    nc.scalar.activation(eAB[:], sAB[:], AF.Exp, scale=0.125)
                    exps.append(eAB)
                    if sk > 0:
                        attnv(sk - 1)
                    want = (sk + 1) * bg_budget // N_SK
                    while drained < want and bg_queue:
                        bg_queue.pop(0)()
                        drained += 1
                attnv(N_SK - 1)

                # normalize: denominators live in row 64 of oTA/oTB.
                # Stage the denom rows to SBUF, broadcast across 64
                # partitions with a K=1 ones outer-product in the bg slot,
                # reciprocal, one multiply per head (head-interleaved).
                oXA = rcp.tile([65, CH], bf16, tag="oX0")
                oXB = rcp.tile([65, CH], bf16, tag="oX1")
                nc.vector.tensor_copy(oXA[64:65, :], oTA[64:65, :])
                nc.vector.tensor_copy(oXB[64:65, :], oTB[64:65, :])
                dbA = bgps.tile([64, CH], f32, tag="bg")
                dbB = bgps.tile([64, CH], f32, tag="bg")
                nc.tensor.matmul(
                    dbA[:], onesrow[64:65, :], oXA[64:65, :], start=True, stop=True
                )
                nc.tensor.matmul(
                    dbB[:], onesrow[64:65, :], oXB[64:65, :], start=True, stop=True
                )
                rbA = rcp.tile([64, CH], f32, tag="rb0")
                rbB = rcp.tile([64, CH], f32, tag="rb1")
                nc.vector.reciprocal_approx_fast(rbA[:], dbA[:])
                nc.vector.reciprocal_approx_fast(rbB[:], dbB[:])
                nc.vector.tensor_mul(att_o[g][0:64, cslice], oTA[0:64, :], rbA[:])
                aoB = rcp.tile([64, CH], bf16, tag="aoB")
                nc.vector.tensor_mul(aoB[:], oTB[0:64, :], rbB[:])
                nc.sync.dma_start(att_o[g][64:128, cslice], aoB[:])

            # ---------------- emission ----------------
            # serial head: pair-0 projection, c-major so the chain pace
            # (~3.4us per xT chunk for q+k) matches the xT chunk DMA
            # arrivals; chains alternate PSUM slots between the bg tag and
            # the still-idle scores tag. Then all of v.
            for i, (c, ti) in enumerate([(c, t) for c in range(NSC) for t in range(2)]):
                for step in chain_steps(0, ti, c, ptag=("bg", "sAB")[i % 2],
                                        copy_eng="scalar"):
                    step()
            while rope_pend:
                rope_tail()
            for st in range(N_SK):
                proj_v(st)

            # pair-0 attention with pair-1 projection as background
            g1_steps = []
            for c in range(NSC):
                for ti in range(2):
                    g1_steps.extend(chain_steps(1, ti, c, copy_eng="vector"))
            for ch in range(N_CH):
                attention_unit(0, ch, g1_steps, (len(g1_steps) + N_CH - 1 - ch) // (N_CH - ch))
            while g1_steps:
                g1_steps.pop(0)()
            while rope_pend:
                rope_tail()

            # pair-1 attention with finished chunks' output projection as
            # background (chunk ch-1 is complete once unit (1, ch-1) done)
            for ch in range(N_CH):
                op_steps = []
                if ch > 0:
                    for st in range(CH * (ch - 1) // 128, CH * ch // 128):
                        op_steps.extend(outproj_steps(st))
                attention_unit(1, ch, op_steps, len(op_steps))
                while op_steps:
                    op_steps.pop(0)()
            for st in range(CH * (N_CH - 1) // 128, S // 128):
                for step in outproj_steps(st, tail=True):
                    step()

    nc.compile()
    return nc


def _get_program():
    if "nc" not in _BUILT:
        _BUILT["nc"] = _build_program()
    return _BUILT["nc"]


def _pack_e(a):
    """[E, C] -> [128, EK*C] with row p = concat over e of a[128e+p, :]."""
    Edim, C = a.shape
    return np.ascontiguousarray(
        a.reshape(EK, 128, C).transpose(1, 0, 2).reshape(128, EK * C)
    )


def _host_inputs(x, W_qkv, W_out):
    """Build the 8 per-core input maps (bf16, DMA-packed)."""
    import ml_dtypes

    bf = ml_dtypes.bfloat16
    f = np.float32
    x = np.asarray(x, dtype=f)
    W_qkv = np.asarray(W_qkv, dtype=f)
    W_out = np.asarray(W_out, dtype=f)

    inv_freq = 1.0 / (ROPE_THETA ** (np.arange(0, D, 2, dtype=np.float64) / D))
    p = np.arange(128)
    freq_row = inv_freq[(p % D) // 2]  # [128]
    ang = freq_row[:, None] * np.arange(S, dtype=np.float64)[None, :]  # [128, S]
    cos_t = np.cos(ang).astype(bf)
    sign = np.where(p % 2 == 0, -1.0, 1.0)[:, None]
    sin_t = (np.sin(ang) * sign).astype(bf)

    msw = np.zeros((128, 128), dtype=f)
    msw[p, p ^ 1] = 1.0

    maps = []
    for core in range(N_CORES):
        b, hg = divmod(core, HG)
        hs = [HPG * hg + i for i in range(HPG)]
        w_qk = np.concatenate(
            [W_qkv[:, h * D : (h + 1) * D] for h in hs]
            + [W_qkv[:, ATT + h * D : ATT + (h + 1) * D] for h in hs],
            axis=1,
        )
        w_v = np.concatenate(
            [W_qkv[:, 2 * ATT + h * D : 2 * ATT + (h + 1) * D] for h in hs], axis=1
        )
        w_o = np.concatenate([W_out[h * D : (h + 1) * D, :] for h in hs], axis=0)
        # wqkm: 4 col-blocks of [128, EK*128] + mswap appended
        blocks = [
            _pack_e(np.ascontiguousarray(w_qk[:, 128 * bb : 128 * (bb + 1)]))
            for bb in range(4)
        ]
        w_qkm = np.concatenate(blocks + [msw], axis=1)
        xT = np.ascontiguousarray(x[b].T)
        m = {
            "w_qkm": w_qkm.astype(bf),
            "w_v": _pack_e(w_v).astype(bf),
            "w_o": np.ascontiguousarray(w_o).astype(bf),
            "cos_t": cos_t,
            "sin_t": sin_t,
        }
        for c in range(NSC):
            m[f"xT{c}"] = _pack_e(xT[:, 512 * c : 512 * (c + 1)]).astype(bf)
        maps.append(m)
    return maps


def _gather(res, inputs=None):
    out = np.zeros((B, S, E), dtype=np.float32)
    for core in range(N_CORES):
        b = core // HG
        out[b] += res.results[core]["out"]
    return out


def kernel(x, W_qkv, W_out):
    from concourse.bass_utils import run_bass_kernel_spmd

    nc = _get_program()
    maps = _host_inputs(x, W_qkv, W_out)
    res = run_bass_kernel_spmd(nc, maps, core_ids=list(range(N_CORES)))
    return _gather(res)
